# revision 5
# baseline (speedup 1.0000x reference)
"""Multi-head attention (shared Wq for Q/K/V projections, Wo output proj)
as a Bass/Tile kernel for 8 Trainium2 NeuronCores.

Problem: B=4, S=2048, D=1024, H=16 heads (dk=64).
  Q = q @ Wq.T ; K = k @ Wq.T ; V = v @ Wq.T   (faithful: Wq for all three)
  out = softmax(Q K^T / 8) V  -> merge heads -> @ Wo.T

Sharding: core c handles batch b=c//2 and head-half half=c%2 (8 heads = 512
projection columns). Each core computes a partial (S, D) output
(head_out_slice @ Wo.T rows) in fp32; host sums the two halves per batch.

Per-core device pipeline (all matmuls bf16, fp32 PSUM accumulate):
  P1 projections:  QT/KT (dims x seq, per head-pair tiles) and V (seq x dims,
                   with a ones column appended per head for softmax denoms).
  P2 attention per (query-block of 512, head-pair):
       MM1: ST units (128 keys, 512 q) = K^T-chunk.T @ Q^T, two heads
            row-packed in the 128x128 PE array (contraction dk=64 each).
       ACT: PT = exp(ST * 1/8) PSUM->SBUF bf16 in N=1536/1024 groups.
       MM2: accum (65, 512) += V_aug[kc].T-style lhsT (128 keys, 64+1) @ PT;
            row 64 (ones column) accumulates the softmax denominator.
       tail: reciprocal of denom row, DMA partition-broadcast (via DRAM
             bounce), normalize rows 0..63 -> head_outT bf16.
  P3 output projection per query-block: accumulate over 4 head-pairs,
     evict fp32, DMA to DRAM.
"""

import numpy as np
import ml_dtypes

BF16 = ml_dtypes.bfloat16

S = 2048          # sequence length
D = 1024          # model dim
COLS = 512        # projection columns per core (8 heads * 64)
P = 128           # SBUF partitions
DK = 64           # head dim
PAIRS = 4         # head pairs per core
KC = S // P       # 16 key chunks
RC = D // P       # 8 contraction chunks for projections
QB = 512          # query block size
NQB = S // QB     # 4 query blocks
N_CORES = 8

_PROGRAM_CACHE = {}


def _emit_kernel(tc, aps):
    import concourse.mybir as mybir

    nc = tc.nc
    f32 = mybir.dt.float32
    bf16 = mybir.dt.bfloat16
    Exp = mybir.ActivationFunctionType.Exp
    mult = mybir.AluOpType.mult

    qT, kT, vT, wq, wo, out = (
        aps["qT"], aps["kT"], aps["vT"], aps["wq"], aps["wo"], aps["out"])

    # head_outT per (pair, qcb): (128 pair-dims, 512 q)
    HOUT = [[None] * NQB for _ in range(PAIRS)]

    with (
        tc.tile_pool(name="persist", bufs=1) as persist,
        tc.tile_pool(name="stage", bufs=4) as stage,        # 4 x 16KB/part
        tc.tile_pool(name="ptp", bufs=8) as ptp,            # exp outputs
        tc.tile_pool(name="hop", bufs=PAIRS * NQB) as hop,  # head_outT tiles
        tc.tile_pool(name="smalls", bufs=2) as smalls,
        tc.tile_pool(name="osbp", bufs=3) as osbp,
        tc.tile_pool(name="dramp", bufs=2, space="DRAM") as dramp,
        tc.tile_pool(name="stps", bufs=1, space="PSUM") as stps,
        tc.tile_pool(name="pbp", bufs=3, space="PSUM") as pbp,
    ):
        # ---------------- persistent SBUF tiles ----------------
        def ptile(shape, name):
            return persist.tile(shape, bf16, tag=name, name=name)

        wq_sb = ptile([P, RC, COLS], "wq_sb")                   # 8 KB/part
        wo_sb = ptile([P, PAIRS, D], "wo_sb")                   # 8 KB/part
        QT = [ptile([P, S], f"QT{p}") for p in range(PAIRS)]
        KT = [ptile([P, S], f"KT{p}") for p in range(PAIRS)]
        # V with ones column per head: (seq part, 8 heads, 64+1)
        V = [ptile([P, 8, DK + 1], f"V{kc}") for kc in range(KC)]

        nc.sync.dma_start(wq_sb[:], wq.rearrange("(r p) n -> p r n", p=P))
        nc.sync.dma_start(wo_sb[:], wo.rearrange("(c p) n -> p c n", p=P))

        # ---------------- P1: projections ----------------
        def load_halves(src):
            halves = []
            for h in range(2):
                t = stage.tile([P, RC // 2, S], bf16, tag="xT",
                               name=f"stg{h}")
                nc.sync.dma_start(
                    t[:], src.rearrange("(r p) n -> p r n", p=P)[:, h * 4:h * 4 + 4, :])
                halves.append(t)
            return halves

        def proj_psum(lhsT_of_rc, rhs_of_rc, n_free):
            ps = pbp.tile([P, QB], f32, tag="pb", name="projps")
            for rc in range(RC):
                hi, r = divmod(rc, 4)
                nc.tensor.matmul(
                    ps[:, :n_free],
                    lhsT_of_rc(hi, r),
                    rhs_of_rc(hi, r),
                    start=(rc == 0), stop=(rc == RC - 1))
            return ps

        # V first (needed by MM2 of every pair)
        v_h = load_halves(vT)
        for kc in range(KC):
            ps = proj_psum(
                lambda hi, r, kc=kc: v_h[hi][:, r, kc * P:(kc + 1) * P],
                lambda hi, r: wq_sb[:, hi * 4 + r, :],
                COLS)
            # evict (128 seq, 512 dims) -> V[kc][:, :, 0:64] (strided by 65)
            nc.vector.tensor_copy(
                out=V[kc][:, :, 0:DK],
                in_=ps.rearrange("p (h d) -> p h d", d=DK))
            nc.vector.memset(V[kc][:, :, DK:DK + 1], 1.0)

        # Q and K per pair (interleaved so pair 0 finishes first)
        q_h = load_halves(qT)
        k_h = load_halves(kT)
        for pair in range(PAIRS):
            for dest, halves in ((QT, q_h), (KT, k_h)):
                for qc in range(NQB):
                    ps = proj_psum(
                        lambda hi, r, pair=pair: wq_sb[:, hi * 4 + r,
                                                       pair * P:(pair + 1) * P],
                        lambda hi, r, qc=qc, hv=halves: hv[hi][:, r, qc * QB:(qc + 1) * QB],
                        QB)
                    nc.vector.tensor_copy(
                        out=dest[pair][:, qc * QB:(qc + 1) * QB], in_=ps[:])

        # ---------------- P2+P3: attention + output projection ----------------
        for qcb in range(NQB):
            q0 = qcb * QB
            for pair in range(PAIRS):
                units = [(j, kc) for kc in range(KC) for j in (0, 1)]
                accum = [
                    pbp.tile([DK + 1, QB], f32, tag="pb", name=f"acc{j}")
                    for j in (0, 1)]
                gi = 0
                ui = 0
                while ui < len(units):
                    cap = 3 if gi % 2 == 0 else 2
                    group = units[ui:ui + cap]
                    n = len(group)
                    tag = "stA" if gi % 2 == 0 else "stB"
                    width = 1536 if gi % 2 == 0 else 1024
                    st = stps.tile([P, width], f32, tag=tag, name="st")
                    # MM1: row-packed pair of heads (j=0 rows 0-63, j=1 rows 64-127)
                    for u, (j, kc) in enumerate(group):
                        nc.tensor.matmul(
                            st[:, u * QB:(u + 1) * QB],
                            KT[pair][j * DK:(j + 1) * DK, kc * P:(kc + 1) * P],
                            QT[pair][j * DK:(j + 1) * DK, q0:q0 + QB],
                            start=True, stop=True)
                    pt = ptp.tile([P, 1536], bf16, tag="pt", name="pt")
                    nc.scalar.activation(
                        pt[:, :n * QB], st[:, :n * QB], Exp, scale=0.125)
                    # MM2: V_aug (64 dims + ones col) x PT -> accum (65, 512)
                    for u, (j, kc) in enumerate(group):
                        nc.tensor.matmul(
                            accum[j][:],
                            V[kc][:, pair * 2 + j, :],
                            pt[:, u * QB:(u + 1) * QB],
                            start=(kc == 0), stop=(kc == KC - 1))
                    ui += n
                    gi += 1
                # tail: normalize by softmax denominator (accum row 64).
                # recip rows live at partitions 0 and 32 (32-aligned bases).
                recip = smalls.tile([33, QB], f32, tag="recip", name="recip")
                rdram = dramp.tile([2, QB], f32, name="rdram")
                for j in (0, 1):
                    nc.vector.reciprocal(
                        recip[j * 32:j * 32 + 1, :], accum[j][DK:DK + 1, :])
                    nc.sync.dma_start(rdram[j:j + 1, :], recip[j * 32:j * 32 + 1, :])
                bcast = smalls.tile([P, QB], f32, tag="bcast", name="bcast")
                for j in (0, 1):
                    nc.sync.dma_start(
                        bcast[j * DK:(j + 1) * DK, :],
                        rdram[j:j + 1, :].to_broadcast((DK, QB)))
                ht = hop.tile([P, QB], bf16, tag="hout", name=f"ht{pair}_{qcb}")
                for j in (0, 1):
                    nc.vector.tensor_tensor(
                        ht[j * DK:(j + 1) * DK, :],
                        accum[j][0:DK, :],
                        bcast[j * DK:(j + 1) * DK, :],
                        mult)
                HOUT[pair][qcb] = ht

            # P3: output projection for this query block
            for qk in range(QB // P):
                osb = osbp.tile([P, D], f32, tag="osb", name="osb")
                for nk in range(2):
                    ps = pbp.tile([P, QB], f32, tag="pb", name="ops")
                    for pair in range(PAIRS):
                        nc.tensor.matmul(
                            ps[:],
                            HOUT[pair][qcb][:, qk * P:(qk + 1) * P],
                            wo_sb[:, pair, nk * QB:(nk + 1) * QB],
                            start=(pair == 0), stop=(pair == PAIRS - 1))
                    nc.vector.tensor_copy(out=osb[:, nk * QB:(nk + 1) * QB], in_=ps[:])
                nc.sync.dma_start(
                    out[q0 + qk * P: q0 + (qk + 1) * P, :], osb[:])


def build_program():
    """Build + compile the single-core SPMD Bass program. Cached per process."""
    if "nc" in _PROGRAM_CACHE:
        return _PROGRAM_CACHE["nc"]
    import concourse.bacc as bacc
    import concourse.tile as tile
    import concourse.mybir as mybir

    bf16 = mybir.dt.bfloat16
    f32 = mybir.dt.float32
    nc = bacc.Bacc("TRN2", target_bir_lowering=False, debug=False)
    aps = {
        "qT": nc.dram_tensor("qT", [D, S], bf16, kind="ExternalInput").ap(),
        "kT": nc.dram_tensor("kT", [D, S], bf16, kind="ExternalInput").ap(),
        "vT": nc.dram_tensor("vT", [D, S], bf16, kind="ExternalInput").ap(),
        "wq": nc.dram_tensor("wq", [D, COLS], bf16, kind="ExternalInput").ap(),
        "wo": nc.dram_tensor("wo", [COLS, D], bf16, kind="ExternalInput").ap(),
        "out": nc.dram_tensor("out", [S, D], f32, kind="ExternalOutput").ap(),
    }
    with tile.TileContext(nc) as tc:
        _emit_kernel(tc, aps)
    nc.compile()
    _PROGRAM_CACHE["nc"] = nc
    return nc


def make_in_maps(q, k, v, Wq, Wo):
    """Host-side sharding: core c -> batch c//2, head-half c%2."""
    q = np.asarray(q, dtype=np.float32)
    k = np.asarray(k, dtype=np.float32)
    v = np.asarray(v, dtype=np.float32)
    Wq = np.asarray(Wq, dtype=np.float32)
    Wo = np.asarray(Wo, dtype=np.float32)
    WqT = np.ascontiguousarray(Wq.T)   # (in D, out D)
    WoT = np.ascontiguousarray(Wo.T)   # (in D, out D)
    in_maps = []
    for c in range(N_CORES):
        b, half = divmod(c, 2)
        cols = slice(half * COLS, (half + 1) * COLS)
        in_maps.append({
            "qT": np.ascontiguousarray(q[b].T).astype(BF16),
            "kT": np.ascontiguousarray(k[b].T).astype(BF16),
            "vT": np.ascontiguousarray(v[b].T).astype(BF16),
            "wq": np.ascontiguousarray(WqT[:, cols]).astype(BF16),
            "wo": np.ascontiguousarray(WoT[cols, :]).astype(BF16),
        })
    return in_maps


def run_cores(in_maps, trace=False, trace_cores=None):
    from concourse.bass_utils import run_bass_kernel_spmd
    nc = build_program()
    return run_bass_kernel_spmd(
        nc, in_maps, core_ids=list(range(N_CORES)),
        trace=trace, trace_cores=trace_cores)


def kernel(q, k, v, Wq, Wo):
    in_maps = make_in_maps(q, k, v, Wq, Wo)
    res = run_cores(in_maps)
    B = 4
    out = np.zeros((B, S, D), dtype=np.float32)
    for c in range(N_CORES):
        out[c // 2] += res.results[c]["out"]
    return out


# revision 8
# speedup vs baseline: 1.0648x; 1.0648x over previous
"""Multi-head attention (shared Wq for Q/K/V projections, Wo output proj)
as a Bass/Tile kernel for 8 Trainium2 NeuronCores.

Problem: B=4, S=2048, D=1024, H=16 heads (dk=64).
  Q = q @ Wq.T ; K = k @ Wq.T ; V = v @ Wq.T   (faithful: Wq for all three)
  out = softmax(Q K^T / 8) V  -> merge heads -> @ Wo.T

Sharding: core c handles batch b=c//2 and head-half half=c%2 (8 heads = 512
projection columns). Each core computes a partial (S, D) output
(head_out_slice @ Wo.T rows) in fp32; host sums the two halves per batch.

Per-core device pipeline (all matmuls bf16, fp32 PSUM accumulate):
  P1 projections:  QT/KT (dims x seq, per head-pair tiles) and V (seq x dims,
                   with a ones column appended per head for softmax denoms).
  P2 attention per (query-block of 512, head-pair):
       MM1: ST units (128 keys, 512 q) = K^T-chunk.T @ Q^T, two heads
            row-packed in the 128x128 PE array (contraction dk=64 each).
       ACT: PT = exp(ST * 1/8) PSUM->SBUF bf16 in N=1536/1024 groups.
       MM2: accum (65, 512) += V_aug[kc].T-style lhsT (128 keys, 64+1) @ PT;
            row 64 (ones column) accumulates the softmax denominator.
       tail: reciprocal of denom row, DMA partition-broadcast (via DRAM
             bounce), normalize rows 0..63 -> head_outT bf16.
  P3 output projection per query-block: accumulate over 4 head-pairs,
     evict fp32, DMA to DRAM.
"""

import numpy as np
import ml_dtypes

BF16 = ml_dtypes.bfloat16

S = 2048          # sequence length
D = 1024          # model dim
COLS = 512        # projection columns per core (8 heads * 64)
P = 128           # SBUF partitions
DK = 64           # head dim
PAIRS = 4         # head pairs per core
KC = S // P       # 16 key chunks
RC = D // P       # 8 contraction chunks for projections
QB = 512          # query block size
NQB = S // QB     # 4 query blocks
N_CORES = 8

_PROGRAM_CACHE = {}


def _emit_kernel(tc, aps):
    import concourse.mybir as mybir

    nc = tc.nc
    f32 = mybir.dt.float32
    bf16 = mybir.dt.bfloat16
    Exp = mybir.ActivationFunctionType.Exp
    mult = mybir.AluOpType.mult

    qT, kT, vT, wq, wo, out = (
        aps["qT"], aps["kT"], aps["vT"], aps["wq"], aps["wo"], aps["out"])

    # head_outT per (pair, qcb): (128 pair-dims, 512 q)
    HOUT = [[None] * NQB for _ in range(PAIRS)]

    with (
        tc.tile_pool(name="persist", bufs=1) as persist,
        tc.tile_pool(name="stage", bufs=4) as stage,        # 4 x 16KB/part
        tc.tile_pool(name="ptp", bufs=8) as ptp,            # exp outputs
        tc.tile_pool(name="hop", bufs=PAIRS * NQB) as hop,  # head_outT tiles
        tc.tile_pool(name="smalls", bufs=2) as smalls,
        tc.tile_pool(name="osbp", bufs=3) as osbp,
        tc.tile_pool(name="dramp", bufs=2, space="DRAM") as dramp,
        tc.tile_pool(name="stps", bufs=1, space="PSUM") as stps,
        tc.tile_pool(name="pbp", bufs=3, space="PSUM") as pbp,
    ):
        # ---------------- persistent SBUF tiles ----------------
        def ptile(shape, name):
            return persist.tile(shape, bf16, tag=name, name=name)

        wq_sb = ptile([P, RC, COLS], "wq_sb")                   # 8 KB/part
        wo_sb = ptile([P, PAIRS, D], "wo_sb")                   # 8 KB/part
        QT = [ptile([P, S], f"QT{p}") for p in range(PAIRS)]
        KT = [ptile([P, S], f"KT{p}") for p in range(PAIRS)]
        # V with ones column per head: (seq part, 8 heads, 64+1)
        V = [ptile([P, 8, DK + 1], f"V{kc}") for kc in range(KC)]

        nc.sync.dma_start(wq_sb[:], wq.rearrange("(r p) n -> p r n", p=P))
        nc.sync.dma_start(wo_sb[:], wo.rearrange("(c p) n -> p c n", p=P))

        # ---------------- P1: projections ----------------
        def load_halves(src):
            halves = []
            for h in range(2):
                t = stage.tile([P, RC // 2, S], bf16, tag="xT",
                               name=f"stg{h}")
                nc.sync.dma_start(
                    t[:], src.rearrange("(r p) n -> p r n", p=P)[:, h * 4:h * 4 + 4, :])
                halves.append(t)
            return halves

        def proj_psum(lhsT_of_rc, rhs_of_rc, n_free):
            ps = pbp.tile([P, QB], f32, tag="pb", name="projps")
            for rc in range(RC):
                hi, r = divmod(rc, 4)
                nc.tensor.matmul(
                    ps[:, :n_free],
                    lhsT_of_rc(hi, r),
                    rhs_of_rc(hi, r),
                    start=(rc == 0), stop=(rc == RC - 1))
            return ps

        # V first (needed by MM2 of every pair)
        v_h = load_halves(vT)
        for kc in range(KC):
            ps = proj_psum(
                lambda hi, r, kc=kc: v_h[hi][:, r, kc * P:(kc + 1) * P],
                lambda hi, r: wq_sb[:, hi * 4 + r, :],
                COLS)
            # evict (128 seq, 512 dims) -> V[kc][:, :, 0:64] (strided by 65)
            nc.vector.tensor_copy(
                out=V[kc][:, :, 0:DK],
                in_=ps.rearrange("p (h d) -> p h d", d=DK))
            nc.vector.memset(V[kc][:, :, DK:DK + 1], 1.0)

        # Q and K per pair (interleaved so pair 0 finishes first)
        q_h = load_halves(qT)
        k_h = load_halves(kT)
        for pair in range(PAIRS):
            for dest, halves in ((QT, q_h), (KT, k_h)):
                for qc in range(NQB):
                    ps = proj_psum(
                        lambda hi, r, pair=pair: wq_sb[:, hi * 4 + r,
                                                       pair * P:(pair + 1) * P],
                        lambda hi, r, qc=qc, hv=halves: hv[hi][:, r, qc * QB:(qc + 1) * QB],
                        QB)
                    nc.vector.tensor_copy(
                        out=dest[pair][:, qc * QB:(qc + 1) * QB], in_=ps[:])

        # ---------------- P2+P3: attention + output projection ----------------
        for qcb in range(NQB):
            q0 = qcb * QB
            for pair in range(PAIRS):
                units = [(j, kc) for kc in range(KC) for j in (0, 1)]
                accum = [
                    pbp.tile([DK + 1, QB], f32, tag="pb", name=f"acc{j}")
                    for j in (0, 1)]
                gi = 0
                ui = 0
                while ui < len(units):
                    cap = 3 if gi % 2 == 0 else 2
                    group = units[ui:ui + cap]
                    n = len(group)
                    tag = "stA" if gi % 2 == 0 else "stB"
                    width = 1536 if gi % 2 == 0 else 1024
                    st = stps.tile([P, width], f32, tag=tag, name="st")
                    # MM1: row-packed pair of heads (j=0 rows 0-63, j=1 rows 64-127)
                    for u, (j, kc) in enumerate(group):
                        nc.tensor.matmul(
                            st[:, u * QB:(u + 1) * QB],
                            KT[pair][j * DK:(j + 1) * DK, kc * P:(kc + 1) * P],
                            QT[pair][j * DK:(j + 1) * DK, q0:q0 + QB],
                            start=True, stop=True)
                    pt = ptp.tile([P, 1536], bf16, tag="pt", name="pt")
                    nc.scalar.activation(
                        pt[:, :n * QB], st[:, :n * QB], Exp, scale=0.125)
                    # MM2: V_aug (64 dims + ones col) x PT -> accum (65, 512)
                    for u, (j, kc) in enumerate(group):
                        nc.tensor.matmul(
                            accum[j][:],
                            V[kc][:, pair * 2 + j, :],
                            pt[:, u * QB:(u + 1) * QB],
                            start=(kc == 0), stop=(kc == KC - 1))
                    ui += n
                    gi += 1
                # Evict accumulators to SBUF immediately: frees the PSUM
                # slots so the next iteration's MM2 is never tail-blocked.
                raw = [smalls.tile([DK + 1, QB], f32, tag="raw", bufs=4,
                                   name=f"raw{j}") for j in (0, 1)]
                for j in (0, 1):
                    nc.vector.tensor_copy(out=raw[j][:], in_=accum[j][:])
                # tail (off critical path): normalize by softmax denominator
                # (raw row 64). recip rows at partitions 0/32 (32-aligned).
                recip = smalls.tile([33, QB], f32, tag="recip", name="recip")
                rdram = dramp.tile([2, QB], f32, name="rdram")
                for j in (0, 1):
                    nc.vector.reciprocal(
                        recip[j * 32:j * 32 + 1, :], raw[j][DK:DK + 1, :])
                    nc.sync.dma_start(rdram[j:j + 1, :], recip[j * 32:j * 32 + 1, :])
                bcast = [smalls.tile([DK, QB], f32, tag="bcast", bufs=4,
                                     name=f"bcast{j}") for j in (0, 1)]
                for j in (0, 1):
                    nc.sync.dma_start(
                        bcast[j][:], rdram[j:j + 1, :].to_broadcast((DK, QB)))
                ht = hop.tile([P, QB], bf16, tag="hout", name=f"ht{pair}_{qcb}")
                for j in (0, 1):
                    nc.vector.tensor_tensor(
                        ht[j * DK:(j + 1) * DK, :],
                        raw[j][0:DK, :],
                        bcast[j][:],
                        mult)
                HOUT[pair][qcb] = ht

            # P3: output projection for this query block
            for qk in range(QB // P):
                osb = osbp.tile([P, D], f32, tag="osb", name="osb")
                for nk in range(2):
                    ps = pbp.tile([P, QB], f32, tag="pb", name="ops")
                    for pair in range(PAIRS):
                        nc.tensor.matmul(
                            ps[:],
                            HOUT[pair][qcb][:, qk * P:(qk + 1) * P],
                            wo_sb[:, pair, nk * QB:(nk + 1) * QB],
                            start=(pair == 0), stop=(pair == PAIRS - 1))
                    nc.vector.tensor_copy(out=osb[:, nk * QB:(nk + 1) * QB], in_=ps[:])
                nc.sync.dma_start(
                    out[q0 + qk * P: q0 + (qk + 1) * P, :], osb[:])


def build_program():
    """Build + compile the single-core SPMD Bass program. Cached per process."""
    if "nc" in _PROGRAM_CACHE:
        return _PROGRAM_CACHE["nc"]
    import concourse.bacc as bacc
    import concourse.tile as tile
    import concourse.mybir as mybir

    bf16 = mybir.dt.bfloat16
    f32 = mybir.dt.float32
    nc = bacc.Bacc("TRN2", target_bir_lowering=False, debug=False)
    aps = {
        "qT": nc.dram_tensor("qT", [D, S], bf16, kind="ExternalInput").ap(),
        "kT": nc.dram_tensor("kT", [D, S], bf16, kind="ExternalInput").ap(),
        "vT": nc.dram_tensor("vT", [D, S], bf16, kind="ExternalInput").ap(),
        "wq": nc.dram_tensor("wq", [D, COLS], bf16, kind="ExternalInput").ap(),
        "wo": nc.dram_tensor("wo", [COLS, D], bf16, kind="ExternalInput").ap(),
        "out": nc.dram_tensor("out", [S, D], f32, kind="ExternalOutput").ap(),
    }
    with tile.TileContext(nc) as tc:
        _emit_kernel(tc, aps)
    nc.compile()
    _PROGRAM_CACHE["nc"] = nc
    return nc


def make_in_maps(q, k, v, Wq, Wo):
    """Host-side sharding: core c -> batch c//2, head-half c%2."""
    q = np.asarray(q, dtype=np.float32)
    k = np.asarray(k, dtype=np.float32)
    v = np.asarray(v, dtype=np.float32)
    Wq = np.asarray(Wq, dtype=np.float32)
    Wo = np.asarray(Wo, dtype=np.float32)
    WqT = np.ascontiguousarray(Wq.T)   # (in D, out D)
    WoT = np.ascontiguousarray(Wo.T)   # (in D, out D)
    in_maps = []
    for c in range(N_CORES):
        b, half = divmod(c, 2)
        cols = slice(half * COLS, (half + 1) * COLS)
        in_maps.append({
            "qT": np.ascontiguousarray(q[b].T).astype(BF16),
            "kT": np.ascontiguousarray(k[b].T).astype(BF16),
            "vT": np.ascontiguousarray(v[b].T).astype(BF16),
            "wq": np.ascontiguousarray(WqT[:, cols]).astype(BF16),
            "wo": np.ascontiguousarray(WoT[cols, :]).astype(BF16),
        })
    return in_maps


def run_cores(in_maps, trace=False, trace_cores=None):
    from concourse.bass_utils import run_bass_kernel_spmd
    nc = build_program()
    return run_bass_kernel_spmd(
        nc, in_maps, core_ids=list(range(N_CORES)),
        trace=trace, trace_cores=trace_cores)


def kernel(q, k, v, Wq, Wo):
    in_maps = make_in_maps(q, k, v, Wq, Wo)
    res = run_cores(in_maps)
    B = 4
    out = np.zeros((B, S, D), dtype=np.float32)
    for c in range(N_CORES):
        out[c // 2] += res.results[c]["out"]
    return out


# revision 10
# speedup vs baseline: 1.1336x; 1.0646x over previous
"""Multi-head attention (shared Wq for Q/K/V projections, Wo output proj)
as a Bass/Tile kernel for 8 Trainium2 NeuronCores.

Problem: B=4, S=2048, D=1024, H=16 heads (dk=64).
  Q = q @ Wq.T ; K = k @ Wq.T ; V = v @ Wq.T   (faithful: Wq for all three)
  out = softmax(Q K^T / 8) V  -> merge heads -> @ Wo.T

Sharding: core c handles batch b=c//2 and head-half half=c%2 (8 heads = 512
projection columns). Each core computes a partial (S, D) output
(head_out_slice @ Wo.T rows) in fp32; host sums the two halves per batch.

Per-core device pipeline (all matmuls bf16, fp32 PSUM accumulate):
  P1 projections:  QT/KT (dims x seq, per head-pair tiles) and V (seq x dims,
                   with a ones column appended per head for softmax denoms).
  P2 attention per (query-block of 512, head-pair):
       MM1: ST units (128 keys, 512 q) = K^T-chunk.T @ Q^T, two heads
            row-packed in the 128x128 PE array (contraction dk=64 each).
       ACT: PT = exp(ST * 1/8) PSUM->SBUF bf16 in N=1536/1024 groups.
       MM2: accum (65, 512) += V_aug[kc].T-style lhsT (128 keys, 64+1) @ PT;
            row 64 (ones column) accumulates the softmax denominator.
       tail: reciprocal of denom row, DMA partition-broadcast (via DRAM
             bounce), normalize rows 0..63 -> head_outT bf16.
  P3 output projection per query-block: accumulate over 4 head-pairs,
     evict fp32, DMA to DRAM.
"""

import numpy as np
import ml_dtypes

BF16 = ml_dtypes.bfloat16

S = 2048          # sequence length
D = 1024          # model dim
COLS = 512        # projection columns per core (8 heads * 64)
P = 128           # SBUF partitions
DK = 64           # head dim
PAIRS = 4         # head pairs per core
KC = S // P       # 16 key chunks
RC = D // P       # 8 contraction chunks for projections
QB = 512          # query block size
NQB = S // QB     # 4 query blocks
N_CORES = 8

_PROGRAM_CACHE = {}


def _emit_kernel(tc, aps):
    import concourse.mybir as mybir

    nc = tc.nc
    f32 = mybir.dt.float32
    bf16 = mybir.dt.bfloat16
    Exp = mybir.ActivationFunctionType.Exp
    mult = mybir.AluOpType.mult

    qT, kT, vT, wq, wo, out = (
        aps["qT"], aps["kT"], aps["vT"], aps["wq"], aps["wo"], aps["out"])

    # head_outT per (pair, qcb): (128 pair-dims, 512 q)
    HOUT = [[None] * NQB for _ in range(PAIRS)]

    with (
        tc.tile_pool(name="persist", bufs=1) as persist,
        tc.tile_pool(name="stage", bufs=4) as stage,        # 4 x 16KB/part
        tc.tile_pool(name="ptp", bufs=10) as ptp,            # exp outputs
        tc.tile_pool(name="hop", bufs=PAIRS * NQB) as hop,  # head_outT tiles
        tc.tile_pool(name="smalls", bufs=2) as smalls,
        tc.tile_pool(name="osbp", bufs=3) as osbp,
        tc.tile_pool(name="dramp", bufs=4, space="DRAM") as dramp,
        tc.tile_pool(name="stps", bufs=1, space="PSUM") as stps,
        tc.tile_pool(name="pbp", bufs=3, space="PSUM") as pbp,
    ):
        # ---------------- persistent SBUF tiles ----------------
        def ptile(shape, name):
            return persist.tile(shape, bf16, tag=name, name=name)

        wq_sb = ptile([P, RC, COLS], "wq_sb")                   # 8 KB/part
        wo_sb = ptile([P, PAIRS, D], "wo_sb")                   # 8 KB/part
        QT = [ptile([P, S], f"QT{p}") for p in range(PAIRS)]
        KT = [ptile([P, S], f"KT{p}") for p in range(PAIRS)]
        # V with ones column per head: (seq part, 8 heads, 64+1)
        V = [ptile([P, 8, DK + 1], f"V{kc}") for kc in range(KC)]

        nc.sync.dma_start(wq_sb[:], wq.rearrange("(r p) n -> p r n", p=P))
        nc.sync.dma_start(wo_sb[:], wo.rearrange("(c p) n -> p c n", p=P))

        # ---------------- P1: projections ----------------
        def load_halves(src):
            halves = []
            for h in range(2):
                t = stage.tile([P, RC // 2, S], bf16, tag="xT",
                               name=f"stg{h}")
                nc.sync.dma_start(
                    t[:], src.rearrange("(r p) n -> p r n", p=P)[:, h * 4:h * 4 + 4, :])
                halves.append(t)
            return halves

        def proj_psum(lhsT_of_rc, rhs_of_rc, n_free):
            ps = pbp.tile([P, QB], f32, tag="pb", name="projps")
            for rc in range(RC):
                hi, r = divmod(rc, 4)
                nc.tensor.matmul(
                    ps[:, :n_free],
                    lhsT_of_rc(hi, r),
                    rhs_of_rc(hi, r),
                    start=(rc == 0), stop=(rc == RC - 1))
            return ps

        # V first (needed by MM2 of every pair)
        v_h = load_halves(vT)
        for kc in range(KC):
            ps = proj_psum(
                lambda hi, r, kc=kc: v_h[hi][:, r, kc * P:(kc + 1) * P],
                lambda hi, r: wq_sb[:, hi * 4 + r, :],
                COLS)
            # evict (128 seq, 512 dims) -> V[kc][:, :, 0:64] (strided by 65)
            nc.vector.tensor_copy(
                out=V[kc][:, :, 0:DK],
                in_=ps.rearrange("p (h d) -> p h d", d=DK))
            nc.vector.memset(V[kc][:, :, DK:DK + 1], 1.0)

        # Q and K per pair (interleaved so pair 0 finishes first)
        q_h = load_halves(qT)
        k_h = load_halves(kT)
        for pair in range(PAIRS):
            for dest, halves in ((QT, q_h), (KT, k_h)):
                for qc in range(NQB):
                    ps = proj_psum(
                        lambda hi, r, pair=pair: wq_sb[:, hi * 4 + r,
                                                       pair * P:(pair + 1) * P],
                        lambda hi, r, qc=qc, hv=halves: hv[hi][:, r, qc * QB:(qc + 1) * QB],
                        QB)
                    nc.vector.tensor_copy(
                        out=dest[pair][:, qc * QB:(qc + 1) * QB], in_=ps[:])

        # ---------------- P2+P3: attention + output projection ----------------
        for qcb in range(NQB):
            q0 = qcb * QB
            for pair in range(PAIRS):
                units = [(j, kc) for kc in range(KC) for j in (0, 1)]
                accum = [
                    pbp.tile([DK + 1, QB], f32, tag="pb", name=f"acc{j}")
                    for j in (0, 1)]
                gi = 0
                ui = 0
                while ui < len(units):
                    cap = 3 if gi % 2 == 0 else 2
                    group = units[ui:ui + cap]
                    n = len(group)
                    tag = "stA" if gi % 2 == 0 else "stB"
                    width = 1536 if gi % 2 == 0 else 1024
                    st = stps.tile([P, width], f32, tag=tag, name="st")
                    # MM1: row-packed pair of heads (j=0 rows 0-63, j=1 rows 64-127)
                    for u, (j, kc) in enumerate(group):
                        nc.tensor.matmul(
                            st[:, u * QB:(u + 1) * QB],
                            KT[pair][j * DK:(j + 1) * DK, kc * P:(kc + 1) * P],
                            QT[pair][j * DK:(j + 1) * DK, q0:q0 + QB],
                            start=True, stop=True)
                    pt = ptp.tile([P, 1536], bf16, tag="pt", name="pt")
                    nc.scalar.activation(
                        pt[:, :n * QB], st[:, :n * QB], Exp, scale=0.125)
                    # MM2: V_aug (64 dims + ones col) x PT -> accum (65, 512)
                    for u, (j, kc) in enumerate(group):
                        nc.tensor.matmul(
                            accum[j][:],
                            V[kc][:, pair * 2 + j, :],
                            pt[:, u * QB:(u + 1) * QB],
                            start=(kc == 0), stop=(kc == KC - 1))
                    ui += n
                    gi += 1
                # Evict accumulators to SBUF immediately: frees the PSUM
                # slots so the next iteration's MM2 is never tail-blocked.
                raw = [smalls.tile([DK + 1, QB], f32, tag="raw", bufs=4,
                                   name=f"raw{j}") for j in (0, 1)]
                for j in (0, 1):
                    nc.vector.tensor_copy(out=raw[j][:], in_=accum[j][:])
                # tail (off critical path): normalize by softmax denominator
                # (raw row 64). DVE reciprocal on a (1, 512) AP would use a
                # single lane (~3.3us); bounce through DRAM and reload as
                # (128, 2, 4) so all 128 lanes divide in parallel.
                rdram = dramp.tile([2, QB], f32, name="rdram")
                for j in (0, 1):
                    nc.sync.dma_start(rdram[j:j + 1, :], raw[j][DK:DK + 1, :])
                rs = smalls.tile([P, 2, QB // P], f32, tag="rs", name="rs")
                nc.sync.dma_start(rs[:], rdram.rearrange("j (p f) -> p j f", p=P))
                rr = smalls.tile([P, 2, QB // P], f32, tag="rr", name="rr")
                nc.vector.reciprocal(rr[:], rs[:])
                rdram2 = dramp.tile([2, QB], f32, name="rdram2")
                nc.sync.dma_start(rdram2.rearrange("j (p f) -> p j f", p=P), rr[:])
                bcast = [smalls.tile([DK, QB], f32, tag="bcast", bufs=4,
                                     name=f"bcast{j}") for j in (0, 1)]
                for j in (0, 1):
                    nc.sync.dma_start(
                        bcast[j][:], rdram2[j:j + 1, :].to_broadcast((DK, QB)))
                ht = hop.tile([P, QB], bf16, tag="hout", name=f"ht{pair}_{qcb}")
                for j in (0, 1):
                    nc.vector.tensor_tensor(
                        ht[j * DK:(j + 1) * DK, :],
                        raw[j][0:DK, :],
                        bcast[j][:],
                        mult)
                HOUT[pair][qcb] = ht

            # P3: output projection for this query block
            for qk in range(QB // P):
                osb = osbp.tile([P, D], f32, tag="osb", name="osb")
                for nk in range(2):
                    ps = pbp.tile([P, QB], f32, tag="pb", name="ops")
                    for pair in range(PAIRS):
                        nc.tensor.matmul(
                            ps[:],
                            HOUT[pair][qcb][:, qk * P:(qk + 1) * P],
                            wo_sb[:, pair, nk * QB:(nk + 1) * QB],
                            start=(pair == 0), stop=(pair == PAIRS - 1))
                    nc.vector.tensor_copy(out=osb[:, nk * QB:(nk + 1) * QB], in_=ps[:])
                nc.sync.dma_start(
                    out[q0 + qk * P: q0 + (qk + 1) * P, :], osb[:])


def build_program():
    """Build + compile the single-core SPMD Bass program. Cached per process."""
    if "nc" in _PROGRAM_CACHE:
        return _PROGRAM_CACHE["nc"]
    import concourse.bacc as bacc
    import concourse.tile as tile
    import concourse.mybir as mybir

    bf16 = mybir.dt.bfloat16
    f32 = mybir.dt.float32
    nc = bacc.Bacc("TRN2", target_bir_lowering=False, debug=False)
    aps = {
        "qT": nc.dram_tensor("qT", [D, S], bf16, kind="ExternalInput").ap(),
        "kT": nc.dram_tensor("kT", [D, S], bf16, kind="ExternalInput").ap(),
        "vT": nc.dram_tensor("vT", [D, S], bf16, kind="ExternalInput").ap(),
        "wq": nc.dram_tensor("wq", [D, COLS], bf16, kind="ExternalInput").ap(),
        "wo": nc.dram_tensor("wo", [COLS, D], bf16, kind="ExternalInput").ap(),
        "out": nc.dram_tensor("out", [S, D], f32, kind="ExternalOutput").ap(),
    }
    with tile.TileContext(nc) as tc:
        _emit_kernel(tc, aps)
    nc.compile()
    _PROGRAM_CACHE["nc"] = nc
    return nc


def make_in_maps(q, k, v, Wq, Wo):
    """Host-side sharding: core c -> batch c//2, head-half c%2."""
    q = np.asarray(q, dtype=np.float32)
    k = np.asarray(k, dtype=np.float32)
    v = np.asarray(v, dtype=np.float32)
    Wq = np.asarray(Wq, dtype=np.float32)
    Wo = np.asarray(Wo, dtype=np.float32)
    WqT = np.ascontiguousarray(Wq.T)   # (in D, out D)
    WoT = np.ascontiguousarray(Wo.T)   # (in D, out D)
    in_maps = []
    for c in range(N_CORES):
        b, half = divmod(c, 2)
        cols = slice(half * COLS, (half + 1) * COLS)
        in_maps.append({
            "qT": np.ascontiguousarray(q[b].T).astype(BF16),
            "kT": np.ascontiguousarray(k[b].T).astype(BF16),
            "vT": np.ascontiguousarray(v[b].T).astype(BF16),
            "wq": np.ascontiguousarray(WqT[:, cols]).astype(BF16),
            "wo": np.ascontiguousarray(WoT[cols, :]).astype(BF16),
        })
    return in_maps


def run_cores(in_maps, trace=False, trace_cores=None):
    from concourse.bass_utils import run_bass_kernel_spmd
    nc = build_program()
    return run_bass_kernel_spmd(
        nc, in_maps, core_ids=list(range(N_CORES)),
        trace=trace, trace_cores=trace_cores)


def kernel(q, k, v, Wq, Wo):
    in_maps = make_in_maps(q, k, v, Wq, Wo)
    res = run_cores(in_maps)
    B = 4
    out = np.zeros((B, S, D), dtype=np.float32)
    for c in range(N_CORES):
        out[c // 2] += res.results[c]["out"]
    return out


# revision 11
# speedup vs baseline: 1.4569x; 1.2852x over previous
"""Multi-head attention (shared Wq for Q/K/V projections, Wo output proj)
as a Bass/Tile kernel for 8 Trainium2 NeuronCores.

Problem: B=4, S=2048, D=1024, H=16 heads (dk=64).
  Q = q @ Wq.T ; K = k @ Wq.T ; V = v @ Wq.T   (faithful: Wq for all three)
  out = softmax(Q K^T / 8) V  -> merge heads -> @ Wo.T

Sharding: core c handles batch b=c//2 and head-half half=c%2 (8 heads = 512
projection columns). Each core computes a partial (S, D) output
(head_out_slice @ Wo.T rows) in fp32; host sums the two halves per batch.

Per-core device pipeline (all matmuls bf16, fp32 PSUM accumulate):
  P1 projections:  QT/KT (dims x seq, per head-pair tiles) and V (seq x dims,
                   with a ones column appended per head for softmax denoms).
  P2 attention per (query-block of 512, head-pair):
       MM1: ST units (128 keys, 512 q) = K^T-chunk.T @ Q^T, two heads
            row-packed in the 128x128 PE array (contraction dk=64 each).
       ACT: PT = exp(ST * 1/8) PSUM->SBUF bf16 in N=1536/1024 groups.
       MM2: accum (65, 512) += V_aug[kc].T-style lhsT (128 keys, 64+1) @ PT;
            row 64 (ones column) accumulates the softmax denominator.
       tail: reciprocal of denom row, DMA partition-broadcast (via DRAM
             bounce), normalize rows 0..63 -> head_outT bf16.
  P3 output projection per query-block: accumulate over 4 head-pairs,
     evict fp32, DMA to DRAM.
"""

import numpy as np
import ml_dtypes

BF16 = ml_dtypes.bfloat16

S = 2048          # sequence length
D = 1024          # model dim
COLS = 512        # projection columns per core (8 heads * 64)
P = 128           # SBUF partitions
DK = 64           # head dim
PAIRS = 4         # head pairs per core
KC = S // P       # 16 key chunks
RC = D // P       # 8 contraction chunks for projections
QB = 512          # query block size
NQB = S // QB     # 4 query blocks
N_CORES = 8

_PROGRAM_CACHE = {}


def _emit_kernel(tc, aps):
    import concourse.mybir as mybir

    nc = tc.nc
    f32 = mybir.dt.float32
    bf16 = mybir.dt.bfloat16
    Exp = mybir.ActivationFunctionType.Exp
    mult = mybir.AluOpType.mult

    qT, kT, vT, wq, wo, out = (
        aps["qT"], aps["kT"], aps["vT"], aps["wq"], aps["wo"], aps["out"])

    # head_outT per (pair, qcb): (128 pair-dims, 512 q)
    HOUT = [[None] * NQB for _ in range(PAIRS)]

    with (
        tc.tile_pool(name="persist", bufs=1) as persist,
        tc.tile_pool(name="stage", bufs=4) as stage,        # 4 x 16KB/part
        tc.tile_pool(name="ptp", bufs=10) as ptp,            # exp outputs
        tc.tile_pool(name="hop", bufs=PAIRS * NQB) as hop,  # head_outT tiles
        tc.tile_pool(name="smalls", bufs=2) as smalls,
        tc.tile_pool(name="osbp", bufs=3) as osbp,
        tc.tile_pool(name="dramp", bufs=4, space="DRAM") as dramp,
        tc.tile_pool(name="stps", bufs=1, space="PSUM") as stps,
        tc.tile_pool(name="pbp", bufs=3, space="PSUM") as pbp,
    ):
        # ---------------- persistent SBUF tiles ----------------
        def ptile(shape, name):
            return persist.tile(shape, bf16, tag=name, name=name)

        wq_sb = ptile([P, RC, COLS], "wq_sb")                   # 8 KB/part
        wo_sb = ptile([P, PAIRS, D], "wo_sb")                   # 8 KB/part
        QT = [ptile([P, S], f"QT{p}") for p in range(PAIRS)]
        KT = [ptile([P, S], f"KT{p}") for p in range(PAIRS)]
        # V with ones column per head: (seq part, 8 heads, 64+1)
        V = [ptile([P, 8, DK + 1], f"V{kc}") for kc in range(KC)]

        nc.sync.dma_start(wq_sb[:], wq.rearrange("(r p) n -> p r n", p=P))
        nc.sync.dma_start(wo_sb[:], wo.rearrange("(c p) n -> p c n", p=P))

        # ---------------- P1: projections ----------------
        def load_halves(src):
            halves = []
            for h in range(2):
                t = stage.tile([P, RC // 2, S], bf16, tag="xT",
                               name=f"stg{h}")
                nc.sync.dma_start(
                    t[:], src.rearrange("(r p) n -> p r n", p=P)[:, h * 4:h * 4 + 4, :])
                halves.append(t)
            return halves

        def proj_psum(lhsT_of_rc, rhs_of_rc, n_free):
            ps = pbp.tile([P, QB], f32, tag="pb", name="projps")
            for rc in range(RC):
                hi, r = divmod(rc, 4)
                nc.tensor.matmul(
                    ps[:, :n_free],
                    lhsT_of_rc(hi, r),
                    rhs_of_rc(hi, r),
                    start=(rc == 0), stop=(rc == RC - 1))
            return ps

        # V first (needed by MM2 of every pair)
        v_h = load_halves(vT)
        for kc in range(KC):
            ps = proj_psum(
                lambda hi, r, kc=kc: v_h[hi][:, r, kc * P:(kc + 1) * P],
                lambda hi, r: wq_sb[:, hi * 4 + r, :],
                COLS)
            # evict (128 seq, 512 dims) -> V[kc][:, :, 0:64] (strided by 65)
            nc.vector.tensor_copy(
                out=V[kc][:, :, 0:DK],
                in_=ps.rearrange("p (h d) -> p h d", d=DK))
            nc.vector.memset(V[kc][:, :, DK:DK + 1], 1.0)

        # Q and K per pair (interleaved so pair 0 finishes first)
        q_h = load_halves(qT)
        k_h = load_halves(kT)
        for pair in range(PAIRS):
            for dest, halves in ((QT, q_h), (KT, k_h)):
                for qc in range(NQB):
                    ps = proj_psum(
                        lambda hi, r, pair=pair: wq_sb[:, hi * 4 + r,
                                                       pair * P:(pair + 1) * P],
                        lambda hi, r, qc=qc, hv=halves: hv[hi][:, r, qc * QB:(qc + 1) * QB],
                        QB)
                    nc.vector.tensor_copy(
                        out=dest[pair][:, qc * QB:(qc + 1) * QB], in_=ps[:])

        # ---------------- P2+P3: attention + output projection ----------------
        # Software pipeline: MM2 consumption trails MM1/exp production by TD
        # groups (globally, across iteration boundaries), so the in-order PE
        # stream never blocks on a just-issued exp. Output-projection work is
        # emitted in small chunks between groups to avoid starving ACT.
        TD = 4

        def emit_tail(pair, qcb, accum):
            # Evict accumulators to SBUF immediately: frees the PSUM slots.
            raw = [smalls.tile([DK + 1, QB], f32, tag="raw", bufs=4,
                               name=f"raw{j}") for j in (0, 1)]
            for j in (0, 1):
                nc.vector.tensor_copy(out=raw[j][:], in_=accum[j][:])
            # Normalize by softmax denominator (raw row 64), off critical
            # path. DVE reciprocal on a (1, 512) AP would use one lane
            # (~3.3us); bounce through DRAM and reload as (128, 2, 4) so all
            # 128 lanes divide in parallel.
            rdram = dramp.tile([2, QB], f32, name="rdram")
            for j in (0, 1):
                nc.sync.dma_start(rdram[j:j + 1, :], raw[j][DK:DK + 1, :])
            rs = smalls.tile([P, 2, QB // P], f32, tag="rs", name="rs")
            nc.sync.dma_start(rs[:], rdram.rearrange("j (p f) -> p j f", p=P))
            rr = smalls.tile([P, 2, QB // P], f32, tag="rr", name="rr")
            nc.vector.reciprocal(rr[:], rs[:])
            rdram2 = dramp.tile([2, QB], f32, name="rdram2")
            nc.sync.dma_start(rdram2.rearrange("j (p f) -> p j f", p=P), rr[:])
            bcast = [smalls.tile([DK, QB], f32, tag="bcast", bufs=4,
                                 name=f"bcast{j}") for j in (0, 1)]
            for j in (0, 1):
                nc.sync.dma_start(
                    bcast[j][:], rdram2[j:j + 1, :].to_broadcast((DK, QB)))
            ht = hop.tile([P, QB], bf16, tag="hout", name=f"ht{pair}_{qcb}")
            for j in (0, 1):
                nc.vector.tensor_tensor(
                    ht[j * DK:(j + 1) * DK, :],
                    raw[j][0:DK, :],
                    bcast[j][:],
                    mult)
            HOUT[pair][qcb] = ht

        def outproj_qk(qcb, qk):
            q0 = qcb * QB
            osb = osbp.tile([P, D], f32, tag="osb", name="osb")
            for nk in range(2):
                ps = pbp.tile([P, QB], f32, tag="pb", name="ops")
                for pair in range(PAIRS):
                    nc.tensor.matmul(
                        ps[:],
                        HOUT[pair][qcb][:, qk * P:(qk + 1) * P],
                        wo_sb[:, pair, nk * QB:(nk + 1) * QB],
                        start=(pair == 0), stop=(pair == PAIRS - 1))
                nc.vector.tensor_copy(out=osb[:, nk * QB:(nk + 1) * QB], in_=ps[:])
            nc.sync.dma_start(
                out[q0 + qk * P: q0 + (qk + 1) * P, :], osb[:])

        pending = []       # deferred MM2 groups: (pair, qcb, pt, group, last)
        iter_accum = {}    # (pair, qcb) -> [accum0, accum1]
        oproj_items = []   # deferred outproj chunks: (qcb, qk)

        def flush_mm2():
            pair, qcb, pt, group, last = pending.pop(0)
            key = (pair, qcb)
            if key not in iter_accum:
                iter_accum[key] = [
                    pbp.tile([DK + 1, QB], f32, tag="pb", name=f"acc{j}")
                    for j in (0, 1)]
            accum = iter_accum[key]
            for u, (j, kc) in enumerate(group):
                nc.tensor.matmul(
                    accum[j][:],
                    V[kc][:, pair * 2 + j, :],
                    pt[:, u * QB:(u + 1) * QB],
                    start=(kc == 0), stop=(kc == KC - 1))
            if last:
                emit_tail(pair, qcb, accum)
                del iter_accum[key]
                if pair == PAIRS - 1:
                    for qk in range(QB // P):
                        oproj_items.append((qcb, qk))

        for qcb in range(NQB):
            q0 = qcb * QB
            for pair in range(PAIRS):
                units = [(j, kc) for kc in range(KC) for j in (0, 1)]
                gi = 0
                ui = 0
                while ui < len(units):
                    cap = 3 if gi % 2 == 0 else 2
                    group = units[ui:ui + cap]
                    n = len(group)
                    tag = "stA" if gi % 2 == 0 else "stB"
                    width = 1536 if gi % 2 == 0 else 1024
                    st = stps.tile([P, width], f32, tag=tag, name="st")
                    # MM1: row-packed head pair (j=0 rows 0-63, j=1 rows 64-127)
                    for u, (j, kc) in enumerate(group):
                        nc.tensor.matmul(
                            st[:, u * QB:(u + 1) * QB],
                            KT[pair][j * DK:(j + 1) * DK, kc * P:(kc + 1) * P],
                            QT[pair][j * DK:(j + 1) * DK, q0:q0 + QB],
                            start=True, stop=True)
                    pt = ptp.tile([P, 1536], bf16, tag="pt", name="pt")
                    nc.scalar.activation(
                        pt[:, :n * QB], st[:, :n * QB], Exp, scale=0.125)
                    pending.append(
                        (pair, qcb, pt, group, ui + n == len(units)))
                    if len(pending) > TD:
                        flush_mm2()
                    if oproj_items:
                        outproj_qk(*oproj_items.pop(0))
                    ui += n
                    gi += 1
        while pending:
            flush_mm2()
        while oproj_items:
            outproj_qk(*oproj_items.pop(0))


def build_program():
    """Build + compile the single-core SPMD Bass program. Cached per process."""
    if "nc" in _PROGRAM_CACHE:
        return _PROGRAM_CACHE["nc"]
    import concourse.bacc as bacc
    import concourse.tile as tile
    import concourse.mybir as mybir

    bf16 = mybir.dt.bfloat16
    f32 = mybir.dt.float32
    nc = bacc.Bacc("TRN2", target_bir_lowering=False, debug=False)
    aps = {
        "qT": nc.dram_tensor("qT", [D, S], bf16, kind="ExternalInput").ap(),
        "kT": nc.dram_tensor("kT", [D, S], bf16, kind="ExternalInput").ap(),
        "vT": nc.dram_tensor("vT", [D, S], bf16, kind="ExternalInput").ap(),
        "wq": nc.dram_tensor("wq", [D, COLS], bf16, kind="ExternalInput").ap(),
        "wo": nc.dram_tensor("wo", [COLS, D], bf16, kind="ExternalInput").ap(),
        "out": nc.dram_tensor("out", [S, D], f32, kind="ExternalOutput").ap(),
    }
    with tile.TileContext(nc) as tc:
        _emit_kernel(tc, aps)
    nc.compile()
    _PROGRAM_CACHE["nc"] = nc
    return nc


def make_in_maps(q, k, v, Wq, Wo):
    """Host-side sharding: core c -> batch c//2, head-half c%2."""
    q = np.asarray(q, dtype=np.float32)
    k = np.asarray(k, dtype=np.float32)
    v = np.asarray(v, dtype=np.float32)
    Wq = np.asarray(Wq, dtype=np.float32)
    Wo = np.asarray(Wo, dtype=np.float32)
    WqT = np.ascontiguousarray(Wq.T)   # (in D, out D)
    WoT = np.ascontiguousarray(Wo.T)   # (in D, out D)
    in_maps = []
    for c in range(N_CORES):
        b, half = divmod(c, 2)
        cols = slice(half * COLS, (half + 1) * COLS)
        in_maps.append({
            "qT": np.ascontiguousarray(q[b].T).astype(BF16),
            "kT": np.ascontiguousarray(k[b].T).astype(BF16),
            "vT": np.ascontiguousarray(v[b].T).astype(BF16),
            "wq": np.ascontiguousarray(WqT[:, cols]).astype(BF16),
            "wo": np.ascontiguousarray(WoT[cols, :]).astype(BF16),
        })
    return in_maps


def run_cores(in_maps, trace=False, trace_cores=None):
    from concourse.bass_utils import run_bass_kernel_spmd
    nc = build_program()
    return run_bass_kernel_spmd(
        nc, in_maps, core_ids=list(range(N_CORES)),
        trace=trace, trace_cores=trace_cores)


def kernel(q, k, v, Wq, Wo):
    in_maps = make_in_maps(q, k, v, Wq, Wo)
    res = run_cores(in_maps)
    B = 4
    out = np.zeros((B, S, D), dtype=np.float32)
    for c in range(N_CORES):
        out[c // 2] += res.results[c]["out"]
    return out


# revision 15
# speedup vs baseline: 1.4621x; 1.0036x over previous
"""Multi-head attention (shared Wq for Q/K/V projections, Wo output proj)
as a Bass/Tile kernel for 8 Trainium2 NeuronCores.

Problem: B=4, S=2048, D=1024, H=16 heads (dk=64).
  Q = q @ Wq.T ; K = k @ Wq.T ; V = v @ Wq.T   (faithful: Wq for all three)
  out = softmax(Q K^T / 8) V  -> merge heads -> @ Wo.T

Sharding: core c handles batch b=c//2 and head-half half=c%2 (8 heads = 512
projection columns). Each core computes a partial (S, D) output
(head_out_slice @ Wo.T rows) in fp32; host sums the two halves per batch.

Per-core device pipeline (all matmuls bf16, fp32 PSUM accumulate):
  P1 projections:  QT/KT (dims x seq, per head-pair tiles) and V (seq x dims,
                   with a ones column appended per head for softmax denoms).
  P2 attention per (query-block of 512, head-pair):
       MM1: ST units (128 keys, 512 q) = K^T-chunk.T @ Q^T, two heads
            row-packed in the 128x128 PE array (contraction dk=64 each).
       ACT: PT = exp(ST * 1/8) PSUM->SBUF bf16 in N=1536/1024 groups.
       MM2: accum (65, 512) += V_aug[kc].T-style lhsT (128 keys, 64+1) @ PT;
            row 64 (ones column) accumulates the softmax denominator.
       tail: reciprocal of denom row, DMA partition-broadcast (via DRAM
             bounce), normalize rows 0..63 -> head_outT bf16.
  P3 output projection per query-block: accumulate over 4 head-pairs,
     evict fp32, DMA to DRAM.
"""

import numpy as np
import ml_dtypes

BF16 = ml_dtypes.bfloat16

S = 2048          # sequence length
D = 1024          # model dim
COLS = 512        # projection columns per core (8 heads * 64)
P = 128           # SBUF partitions
DK = 64           # head dim
PAIRS = 4         # head pairs per core
KC = S // P       # 16 key chunks
RC = D // P       # 8 contraction chunks for projections
QB = 512          # query block size
NQB = S // QB     # 4 query blocks
N_CORES = 8

_PROGRAM_CACHE = {}


def _emit_kernel(tc, aps):
    import concourse.mybir as mybir

    nc = tc.nc
    f32 = mybir.dt.float32
    bf16 = mybir.dt.bfloat16
    Exp = mybir.ActivationFunctionType.Exp
    mult = mybir.AluOpType.mult

    qT, kT, vT, wq, wo, out = (
        aps["qT"], aps["kT"], aps["vT"], aps["wq"], aps["wo"], aps["out"])

    # head_outT per (pair, qcb): (128 pair-dims, 512 q)
    HOUT = [[None] * NQB for _ in range(PAIRS)]

    with (
        tc.tile_pool(name="persist", bufs=1) as persist,
        tc.tile_pool(name="stage", bufs=4) as stage,        # 4 x 16KB/part
        tc.tile_pool(name="ptp", bufs=8) as ptp,            # exp outputs
        tc.tile_pool(name="hop", bufs=12) as hop,  # head_outT tiles
        tc.tile_pool(name="smalls", bufs=2) as smalls,
        tc.tile_pool(name="osbp", bufs=2) as osbp,
        tc.tile_pool(name="dramp", bufs=4, space="DRAM") as dramp,
        tc.tile_pool(name="stps", bufs=1, space="PSUM") as stps,
        tc.tile_pool(name="pbp", bufs=3, space="PSUM") as pbp,
    ):
        # ---------------- persistent SBUF tiles ----------------
        def ptile(shape, name):
            return persist.tile(shape, bf16, tag=name, name=name)

        wq_sb = ptile([P, RC, COLS], "wq_sb")                   # 8 KB/part
        wo_sb = ptile([P, PAIRS, D], "wo_sb")                   # 8 KB/part
        QT = [ptile([P, S], f"QT{p}") for p in range(PAIRS)]
        KT = [ptile([P, S], f"KT{p}") for p in range(PAIRS)]
        # V with ones column per head: (seq part, 8 heads, 64+1)
        V = [ptile([P, 8, DK + 1], f"V{kc}") for kc in range(KC)]

        nc.sync.dma_start(wq_sb[:], wq.rearrange("(r p) n -> p r n", p=P))
        nc.sync.dma_start(wo_sb[:], wo.rearrange("(c p) n -> p c n", p=P))

        # ---------------- P1: projections (chunked staging) ----------------
        # Stage (128, 8, 512) column-chunks of the transposed inputs on
        # demand. Only V, KT[0] and QT[0][qc0] are projected up front; the
        # remaining 27 projection tiles are deadline-ordered background work
        # pumped into the attention stream (PE has slack under the ACT-bound
        # exp pipeline).
        kch = [None] * NQB
        qch = [None] * NQB

        def chunk(cache, src, qc, tagn):
            if cache[qc] is None:
                t = stage.tile([P, RC, QB], bf16, tag=tagn, bufs=4,
                               name=f"{tagn}{qc}")
                nc.sync.dma_start(
                    t[:],
                    src.rearrange("(r p) n -> p r n", p=P)[:, :, qc * QB:(qc + 1) * QB])
                cache[qc] = t
            return cache[qc]

        def proj_psum(lhsT_of_rc, rhs_of_rc, n_free):
            ps = pbp.tile([P, QB], f32, tag="pb", name="projps")
            for rc in range(RC):
                nc.tensor.matmul(
                    ps[:, :n_free], lhsT_of_rc(rc), rhs_of_rc(rc),
                    start=(rc == 0), stop=(rc == RC - 1))
            return ps

        def proj_v(qc):
            vc = stage.tile([P, RC, QB], bf16, tag="vc", bufs=2, name=f"vc{qc}")
            nc.sync.dma_start(
                vc[:],
                vT.rearrange("(r p) n -> p r n", p=P)[:, :, qc * QB:(qc + 1) * QB])
            for k4 in range(4):
                kc = qc * 4 + k4
                ps = proj_psum(
                    lambda rc, k4=k4: vc[:, rc, k4 * P:(k4 + 1) * P],
                    lambda rc: wq_sb[:, rc, :],
                    COLS)
                nc.vector.tensor_copy(
                    out=V[kc][:, :, 0:DK],
                    in_=ps.rearrange("p (h d) -> p h d", d=DK))
                nc.vector.memset(V[kc][:, :, DK:DK + 1], 1.0)

        def proj_k(pair, qc):
            t = chunk(kch, kT, qc, "kc")
            ps = proj_psum(
                lambda rc: wq_sb[:, rc, pair * P:(pair + 1) * P],
                lambda rc: t[:, rc, :],
                QB)
            nc.vector.tensor_copy(
                out=KT[pair][:, qc * QB:(qc + 1) * QB], in_=ps[:])

        def proj_q(pair, qc):
            t = chunk(qch, qT, qc, "qc")
            ps = proj_psum(
                lambda rc: wq_sb[:, rc, pair * P:(pair + 1) * P],
                lambda rc: t[:, rc, :],
                QB)
            nc.vector.tensor_copy(
                out=QT[pair][:, qc * QB:(qc + 1) * QB], in_=ps[:])

        # head: V, KT[0], QT[0][qc0]
        for qc in range(NQB):
            proj_v(qc)
        for qc in range(NQB):
            proj_k(0, qc)
        proj_q(0, 0)

        # background projection items, ordered by first-use deadline
        bg = []
        for it in ["K1q0", "Q1q0", "K1q1", "K2q0", "K1q2", "Q2q0", "K1q3",
                   "K2q1", "K3q0", "K2q2", "Q3q0", "K2q3", "K3q1", "K3q2",
                   "K3q3"]:
            fn = proj_k if it[0] == "K" else proj_q
            bg.append((fn, int(it[1]), int(it[3])))
        for qc in range(1, NQB):
            for pair in range(PAIRS):
                bg.append((proj_q, pair, qc))

        def pump_bg():
            if bg:
                fn, pair, qc = bg.pop(0)
                fn(pair, qc)

        # ---------------- P2+P3: attention + output projection ----------------
        # Software pipeline: MM2 consumption trails MM1/exp production by TD
        # groups (globally, across iteration boundaries), so the in-order PE
        # stream never blocks on a just-issued exp. Output-projection work is
        # emitted in small chunks between groups to avoid starving ACT.
        TD = 4

        def emit_tail(pair, qcb, accum):
            # Evict accumulators to SBUF immediately: frees the PSUM slots.
            raw = [smalls.tile([DK + 1, QB], f32, tag="raw", bufs=4,
                               name=f"raw{j}") for j in (0, 1)]
            for j in (0, 1):
                nc.vector.tensor_copy(out=raw[j][:], in_=accum[j][:])
            # Normalize by softmax denominator (raw row 64), off critical
            # path. DVE reciprocal on a (1, 512) AP would use one lane
            # (~3.3us); bounce through DRAM and reload as (128, 2, 4) so all
            # 128 lanes divide in parallel.
            rdram = dramp.tile([2, QB], f32, name="rdram")
            for j in (0, 1):
                nc.sync.dma_start(rdram[j:j + 1, :], raw[j][DK:DK + 1, :])
            rs = smalls.tile([P, 2, QB // P], f32, tag="rs", name="rs")
            nc.sync.dma_start(rs[:], rdram.rearrange("j (p f) -> p j f", p=P))
            rr = smalls.tile([P, 2, QB // P], f32, tag="rr", name="rr")
            nc.vector.reciprocal(rr[:], rs[:])
            rdram2 = dramp.tile([2, QB], f32, name="rdram2")
            nc.sync.dma_start(rdram2.rearrange("j (p f) -> p j f", p=P), rr[:])
            bcast = [smalls.tile([DK, QB], f32, tag="bcast", bufs=4,
                                 name=f"bcast{j}") for j in (0, 1)]
            for j in (0, 1):
                nc.sync.dma_start(
                    bcast[j][:], rdram2[j:j + 1, :].to_broadcast((DK, QB)))
            ht = hop.tile([P, QB], bf16, tag="hout", name=f"ht{pair}_{qcb}")
            for j in (0, 1):
                nc.vector.tensor_tensor(
                    ht[j * DK:(j + 1) * DK, :],
                    raw[j][0:DK, :],
                    bcast[j][:],
                    mult)
            HOUT[pair][qcb] = ht

        def outproj_qk(qcb, qk):
            q0 = qcb * QB
            osb = osbp.tile([P, D], f32, tag="osb", name="osb")
            for nk in range(2):
                ps = pbp.tile([P, QB], f32, tag="pb", name="ops")
                for pair in range(PAIRS):
                    nc.tensor.matmul(
                        ps[:],
                        HOUT[pair][qcb][:, qk * P:(qk + 1) * P],
                        wo_sb[:, pair, nk * QB:(nk + 1) * QB],
                        start=(pair == 0), stop=(pair == PAIRS - 1))
                nc.vector.tensor_copy(out=osb[:, nk * QB:(nk + 1) * QB], in_=ps[:])
            nc.sync.dma_start(
                out[q0 + qk * P: q0 + (qk + 1) * P, :], osb[:])

        pending = []       # deferred MM2 groups: (pair, qcb, pt, group, last)
        iter_accum = {}    # (pair, qcb) -> [accum0, accum1]
        oproj_items = []   # deferred outproj chunks: (qcb, qk)

        def flush_mm2():
            pair, qcb, pt, group, last = pending.pop(0)
            key = (pair, qcb)
            if key not in iter_accum:
                iter_accum[key] = [
                    pbp.tile([DK + 1, QB], f32, tag="pb", name=f"acc{j}")
                    for j in (0, 1)]
            accum = iter_accum[key]
            for u, (j, kc) in enumerate(group):
                nc.tensor.matmul(
                    accum[j][:],
                    V[kc][:, pair * 2 + j, :],
                    pt[:, u * QB:(u + 1) * QB],
                    start=(kc == 0), stop=(kc == KC - 1))
            if last:
                emit_tail(pair, qcb, accum)
                del iter_accum[key]
                if pair == PAIRS - 1:
                    for qk in range(QB // P):
                        oproj_items.append((qcb, qk))

        for qcb in range(NQB):
            q0 = qcb * QB
            for pair in range(PAIRS):
                units = [(j, kc) for kc in range(KC) for j in (0, 1)]
                gi = 0
                ui = 0
                while ui < len(units):
                    cap = 3 if gi % 2 == 0 else 2
                    group = units[ui:ui + cap]
                    n = len(group)
                    tag = "stA" if gi % 2 == 0 else "stB"
                    width = 1536 if gi % 2 == 0 else 1024
                    st = stps.tile([P, width], f32, tag=tag, name="st")
                    # MM1: row-packed head pair (j=0 rows 0-63, j=1 rows 64-127)
                    for u, (j, kc) in enumerate(group):
                        nc.tensor.matmul(
                            st[:, u * QB:(u + 1) * QB],
                            KT[pair][j * DK:(j + 1) * DK, kc * P:(kc + 1) * P],
                            QT[pair][j * DK:(j + 1) * DK, q0:q0 + QB],
                            start=True, stop=True)
                    pt = ptp.tile([P, 1536], bf16, tag="pt", name="pt")
                    nc.scalar.activation(
                        pt[:, :n * QB], st[:, :n * QB], Exp, scale=0.125)
                    pending.append(
                        (pair, qcb, pt, group, ui + n == len(units)))
                    if len(pending) > TD:
                        flush_mm2()
                    if oproj_items:
                        outproj_qk(*oproj_items.pop(0))
                    elif gi % 2 == 1:
                        pump_bg()
                    ui += n
                    gi += 1
        while pending:
            flush_mm2()
        while oproj_items:
            outproj_qk(*oproj_items.pop(0))


def build_program():
    """Build + compile the single-core SPMD Bass program. Cached per process."""
    if "nc" in _PROGRAM_CACHE:
        return _PROGRAM_CACHE["nc"]
    import concourse.bacc as bacc
    import concourse.tile as tile
    import concourse.mybir as mybir

    bf16 = mybir.dt.bfloat16
    f32 = mybir.dt.float32
    nc = bacc.Bacc("TRN2", target_bir_lowering=False, debug=False)
    aps = {
        "qT": nc.dram_tensor("qT", [D, S], bf16, kind="ExternalInput").ap(),
        "kT": nc.dram_tensor("kT", [D, S], bf16, kind="ExternalInput").ap(),
        "vT": nc.dram_tensor("vT", [D, S], bf16, kind="ExternalInput").ap(),
        "wq": nc.dram_tensor("wq", [D, COLS], bf16, kind="ExternalInput").ap(),
        "wo": nc.dram_tensor("wo", [COLS, D], bf16, kind="ExternalInput").ap(),
        "out": nc.dram_tensor("out", [S, D], f32, kind="ExternalOutput").ap(),
    }
    with tile.TileContext(nc) as tc:
        _emit_kernel(tc, aps)
    nc.compile()
    _PROGRAM_CACHE["nc"] = nc
    return nc


def make_in_maps(q, k, v, Wq, Wo):
    """Host-side sharding: core c -> batch c//2, head-half c%2."""
    q = np.asarray(q, dtype=np.float32)
    k = np.asarray(k, dtype=np.float32)
    v = np.asarray(v, dtype=np.float32)
    Wq = np.asarray(Wq, dtype=np.float32)
    Wo = np.asarray(Wo, dtype=np.float32)
    WqT = np.ascontiguousarray(Wq.T)   # (in D, out D)
    WoT = np.ascontiguousarray(Wo.T)   # (in D, out D)
    in_maps = []
    for c in range(N_CORES):
        b, half = divmod(c, 2)
        cols = slice(half * COLS, (half + 1) * COLS)
        in_maps.append({
            "qT": np.ascontiguousarray(q[b].T).astype(BF16),
            "kT": np.ascontiguousarray(k[b].T).astype(BF16),
            "vT": np.ascontiguousarray(v[b].T).astype(BF16),
            "wq": np.ascontiguousarray(WqT[:, cols]).astype(BF16),
            "wo": np.ascontiguousarray(WoT[cols, :]).astype(BF16),
        })
    return in_maps


def run_cores(in_maps, trace=False, trace_cores=None):
    from concourse.bass_utils import run_bass_kernel_spmd
    nc = build_program()
    return run_bass_kernel_spmd(
        nc, in_maps, core_ids=list(range(N_CORES)),
        trace=trace, trace_cores=trace_cores)


def kernel(q, k, v, Wq, Wo):
    in_maps = make_in_maps(q, k, v, Wq, Wo)
    res = run_cores(in_maps)
    B = 4
    out = np.zeros((B, S, D), dtype=np.float32)
    for c in range(N_CORES):
        out[c // 2] += res.results[c]["out"]
    return out


# revision 18
# speedup vs baseline: 1.5081x; 1.0315x over previous
"""Multi-head attention (shared Wq for Q/K/V projections, Wo output proj)
as a Bass/Tile kernel for 8 Trainium2 NeuronCores.

Problem: B=4, S=2048, D=1024, H=16 heads (dk=64).
  Q = q @ Wq.T ; K = k @ Wq.T ; V = v @ Wq.T   (faithful: Wq for all three)
  out = softmax(Q K^T / 8) V  -> merge heads -> @ Wo.T

Sharding: core c handles batch b=c//2 and head-half half=c%2 (8 heads = 512
projection columns). Each core computes a partial (S, D) output
(head_out_slice @ Wo.T rows) in fp32; host sums the two halves per batch.

Per-core device pipeline (all matmuls bf16, fp32 PSUM accumulate):
  P1 projections:  QT/KT (dims x seq, per head-pair tiles) and V (seq x dims,
                   with a ones column appended per head for softmax denoms).
  P2 attention per (query-block of 512, head-pair):
       MM1: ST units (128 keys, 512 q) = K^T-chunk.T @ Q^T, two heads
            row-packed in the 128x128 PE array (contraction dk=64 each).
       ACT: PT = exp(ST * 1/8) PSUM->SBUF bf16 in N=1536/1024 groups.
       MM2: accum (65, 512) += V_aug[kc].T-style lhsT (128 keys, 64+1) @ PT;
            row 64 (ones column) accumulates the softmax denominator.
       tail: reciprocal of denom row, DMA partition-broadcast (via DRAM
             bounce), normalize rows 0..63 -> head_outT bf16.
  P3 output projection per query-block: accumulate over 4 head-pairs,
     evict fp32, DMA to DRAM.
"""

import numpy as np
import ml_dtypes

BF16 = ml_dtypes.bfloat16

S = 2048          # sequence length
D = 1024          # model dim
COLS = 512        # projection columns per core (8 heads * 64)
P = 128           # SBUF partitions
DK = 64           # head dim
PAIRS = 4         # head pairs per core
KC = S // P       # 16 key chunks
RC = D // P       # 8 contraction chunks for projections
QB = 512          # query block size
NQB = S // QB     # 4 query blocks
N_CORES = 8

_PROGRAM_CACHE = {}


def _emit_kernel(tc, aps):
    import concourse.mybir as mybir

    nc = tc.nc
    f32 = mybir.dt.float32
    bf16 = mybir.dt.bfloat16
    Exp = mybir.ActivationFunctionType.Exp
    mult = mybir.AluOpType.mult

    qT, kT, vT, wq, wo, out = (
        aps["qT"], aps["kT"], aps["vT"], aps["wq"], aps["wo"], aps["out"])

    # head_outT per (pair, qcb): (128 pair-dims, 512 q)
    HOUT = [[None] * NQB for _ in range(PAIRS)]

    with (
        tc.tile_pool(name="persist", bufs=1) as persist,
        tc.tile_pool(name="stage", bufs=4) as stage,        # 4 x 16KB/part
        tc.tile_pool(name="ptp", bufs=8) as ptp,            # exp outputs
        tc.tile_pool(name="hop", bufs=12) as hop,  # head_outT tiles
        tc.tile_pool(name="smalls", bufs=2) as smalls,
        tc.tile_pool(name="osbp", bufs=2) as osbp,
        tc.tile_pool(name="dramp", bufs=4, space="DRAM") as dramp,
        tc.tile_pool(name="stps", bufs=1, space="PSUM") as stps,
        tc.tile_pool(name="pbp", bufs=3, space="PSUM") as pbp,
    ):
        # ---------------- persistent SBUF tiles ----------------
        def ptile(shape, name):
            return persist.tile(shape, bf16, tag=name, name=name)

        wq_sb = ptile([P, RC, COLS], "wq_sb")                   # 8 KB/part
        wo_sb = ptile([P, PAIRS, D], "wo_sb")                   # 8 KB/part
        QT = [ptile([P, S], f"QT{p}") for p in range(PAIRS)]
        KT = [ptile([P, S], f"KT{p}") for p in range(PAIRS)]
        # V with ones column per head: (seq part, 8 heads, 64+1)
        V = [ptile([P, 8, DK + 1], f"V{kc}") for kc in range(KC)]

        nc.sync.dma_start(wq_sb[:], wq.rearrange("(r p) n -> p r n", p=P))
        nc.sync.dma_start(wo_sb[:], wo.rearrange("(c p) n -> p c n", p=P))

        # ---------------- P1: projections (chunked staging) ----------------
        # Stage (128, 8, 512) column-chunks of the transposed inputs on
        # demand. Only V, KT[0] and QT[0][qc0] are projected up front; the
        # remaining 27 projection tiles are deadline-ordered background work
        # pumped into the attention stream (PE has slack under the ACT-bound
        # exp pipeline).
        kch = [None] * NQB
        qch = [None] * NQB

        def chunk(cache, src, qc, tagn):
            if cache[qc] is None:
                t = stage.tile([P, RC, QB], bf16, tag=tagn, bufs=4,
                               name=f"{tagn}{qc}")
                nc.sync.dma_start(
                    t[:],
                    src.rearrange("(r p) n -> p r n", p=P)[:, :, qc * QB:(qc + 1) * QB])
                cache[qc] = t
            return cache[qc]

        def proj_psum(lhsT_of_rc, rhs_of_rc, n_free):
            ps = pbp.tile([P, QB], f32, tag="pb", name="projps")
            for rc in range(RC):
                nc.tensor.matmul(
                    ps[:, :n_free], lhsT_of_rc(rc), rhs_of_rc(rc),
                    start=(rc == 0), stop=(rc == RC - 1))
            return ps

        def proj_v(qc):
            vc = stage.tile([P, RC, QB], bf16, tag="vc", bufs=2, name=f"vc{qc}")
            nc.sync.dma_start(
                vc[:],
                vT.rearrange("(r p) n -> p r n", p=P)[:, :, qc * QB:(qc + 1) * QB])
            for k4 in range(4):
                kc = qc * 4 + k4
                ps = proj_psum(
                    lambda rc, k4=k4: vc[:, rc, k4 * P:(k4 + 1) * P],
                    lambda rc: wq_sb[:, rc, :],
                    COLS)
                nc.vector.tensor_copy(
                    out=V[kc][:, :, 0:DK],
                    in_=ps.rearrange("p (h d) -> p h d", d=DK))
                nc.vector.memset(V[kc][:, :, DK:DK + 1], 1.0)

        def proj_k(pair, qc):
            t = chunk(kch, kT, qc, "kc")
            ps = proj_psum(
                lambda rc: wq_sb[:, rc, pair * P:(pair + 1) * P],
                lambda rc: t[:, rc, :],
                QB)
            nc.vector.tensor_copy(
                out=KT[pair][:, qc * QB:(qc + 1) * QB], in_=ps[:])

        def proj_q(pair, qc):
            t = chunk(qch, qT, qc, "qc")
            ps = proj_psum(
                lambda rc: wq_sb[:, rc, pair * P:(pair + 1) * P],
                lambda rc: t[:, rc, :],
                QB)
            nc.vector.tensor_copy(
                out=QT[pair][:, qc * QB:(qc + 1) * QB], in_=ps[:])

        # head: V, KT[0], QT[0][qc0]
        for qc in range(NQB):
            proj_v(qc)
        for qc in range(NQB):
            proj_k(0, qc)
        proj_q(0, 0)

        # background projection items, ordered by first-use deadline
        bg = []
        for it in ["K1q0", "Q1q0", "K1q1", "K2q0", "K1q2", "Q2q0", "K1q3",
                   "K2q1", "K3q0", "K2q2", "Q3q0", "K2q3", "K3q1", "K3q2",
                   "K3q3"]:
            fn = proj_k if it[0] == "K" else proj_q
            bg.append((fn, int(it[1]), int(it[3])))
        for qc in range(1, NQB):
            for pair in range(PAIRS):
                bg.append((proj_q, pair, qc))

        def pump_bg():
            if bg:
                fn, pair, qc = bg.pop(0)
                fn(pair, qc)

        # ---------------- P2+P3: attention + output projection ----------------
        # Software pipeline: MM2 consumption trails MM1/exp production by TD
        # groups (globally, across iteration boundaries), so the in-order PE
        # stream never blocks on a just-issued exp. Output-projection work is
        # emitted in small chunks between groups to avoid starving ACT.
        TD = 4

        def emit_tail(pair, qcb, accum):
            # Evict accumulators to SBUF immediately: frees the PSUM slots.
            raw = [smalls.tile([DK + 1, QB], f32, tag="raw", bufs=4,
                               name=f"raw{j}") for j in (0, 1)]
            for j in (0, 1):
                nc.vector.tensor_copy(out=raw[j][:], in_=accum[j][:])
            # Normalize by softmax denominator (raw row 64), off critical
            # path. DVE reciprocal on a (1, 512) AP would use one lane
            # (~3.3us); bounce through DRAM and reload as (128, 2, 4) so all
            # 128 lanes divide in parallel.
            rdram = dramp.tile([2, QB], f32, name="rdram")
            for j in (0, 1):
                nc.sync.dma_start(rdram[j:j + 1, :], raw[j][DK:DK + 1, :])
            rs = smalls.tile([P, 2, QB // P], f32, tag="rs", name="rs")
            nc.sync.dma_start(rs[:], rdram.rearrange("j (p f) -> p j f", p=P))
            rr = smalls.tile([P, 2, QB // P], f32, tag="rr", name="rr")
            nc.vector.reciprocal(rr[:], rs[:])
            rdram2 = dramp.tile([2, QB], f32, name="rdram2")
            nc.sync.dma_start(rdram2.rearrange("j (p f) -> p j f", p=P), rr[:])
            bcast = [smalls.tile([DK, QB], f32, tag="bcast", bufs=4,
                                 name=f"bcast{j}") for j in (0, 1)]
            for j in (0, 1):
                nc.sync.dma_start(
                    bcast[j][:], rdram2[j:j + 1, :].to_broadcast((DK, QB)))
            ht = hop.tile([P, QB], bf16, tag="hout", name=f"ht{pair}_{qcb}")
            for j in (0, 1):
                nc.vector.tensor_tensor(
                    ht[j * DK:(j + 1) * DK, :],
                    raw[j][0:DK, :],
                    bcast[j][:],
                    mult)
            HOUT[pair][qcb] = ht

        def outproj_qk(qcb, qk):
            q0 = qcb * QB
            osb = osbp.tile([P, D], f32, tag="osb", name="osb")
            for nk in range(2):
                ps = pbp.tile([P, QB], f32, tag="pb", name="ops")
                for pair in range(PAIRS):
                    nc.tensor.matmul(
                        ps[:],
                        HOUT[pair][qcb][:, qk * P:(qk + 1) * P],
                        wo_sb[:, pair, nk * QB:(nk + 1) * QB],
                        start=(pair == 0), stop=(pair == PAIRS - 1))
                nc.vector.tensor_copy(out=osb[:, nk * QB:(nk + 1) * QB], in_=ps[:])
            nc.sync.dma_start(
                out[q0 + qk * P: q0 + (qk + 1) * P, :], osb[:])

        pending = []       # deferred MM2 groups: (pair, qcb, pt, group, last)
        iter_accum = {}    # (pair, qcb) -> [accum0, accum1]
        oproj_items = []   # deferred outproj chunks: (ready_at_gc, qcb, qk)
        gc_box = [0]       # global group counter

        def flush_mm2():
            pair, qcb, pt, group, last = pending.pop(0)
            key = (pair, qcb)
            if key not in iter_accum:
                iter_accum[key] = [
                    pbp.tile([DK + 1, QB], f32, tag="pb", name=f"acc{j}")
                    for j in (0, 1)]
            accum = iter_accum[key]
            for u, (j, kc) in enumerate(group):
                nc.tensor.matmul(
                    accum[j][:],
                    V[kc][:, pair * 2 + j, :],
                    pt[:, u * QB:(u + 1) * QB],
                    start=(kc == 0), stop=(kc == KC - 1))
            if last:
                emit_tail(pair, qcb, accum)
                del iter_accum[key]
                if pair == PAIRS - 1:
                    # The normalize chain (reciprocal via DRAM bounce +
                    # broadcast) takes ~8us; don't let the in-order PE stream
                    # hit outproj matmuls before head_outT can possibly be
                    # ready, or the whole pipeline stalls head-of-line.
                    for qk in range(QB // P):
                        oproj_items.append((gc_box[0] + 10 + qk, qcb, qk))

        for qcb in range(NQB):
            q0 = qcb * QB
            for pair in range(PAIRS):
                units = [(j, kc) for kc in range(KC) for j in (0, 1)]
                gi = 0
                ui = 0
                while ui < len(units):
                    cap = 3 if gi % 2 == 0 else 2
                    group = units[ui:ui + cap]
                    n = len(group)
                    tag = "stA" if gi % 2 == 0 else "stB"
                    width = 1536 if gi % 2 == 0 else 1024
                    st = stps.tile([P, width], f32, tag=tag, name="st")
                    # MM1: row-packed head pair (j=0 rows 0-63, j=1 rows 64-127)
                    for u, (j, kc) in enumerate(group):
                        nc.tensor.matmul(
                            st[:, u * QB:(u + 1) * QB],
                            KT[pair][j * DK:(j + 1) * DK, kc * P:(kc + 1) * P],
                            QT[pair][j * DK:(j + 1) * DK, q0:q0 + QB],
                            start=True, stop=True)
                    pt = ptp.tile([P, 1536], bf16, tag="pt", name="pt")
                    nc.scalar.activation(
                        pt[:, :n * QB], st[:, :n * QB], Exp, scale=0.125)
                    pending.append(
                        (pair, qcb, pt, group, ui + n == len(units)))
                    if len(pending) > TD:
                        flush_mm2()
                    gc_box[0] += 1
                    if oproj_items and oproj_items[0][0] <= gc_box[0]:
                        _, oq, ok = oproj_items.pop(0)
                        outproj_qk(oq, ok)
                    elif gi % 2 == 1:
                        pump_bg()
                    ui += n
                    gi += 1
        while pending:
            flush_mm2()
        while oproj_items:
            _, oq, ok = oproj_items.pop(0)
            outproj_qk(oq, ok)


def build_program():
    """Build + compile the single-core SPMD Bass program. Cached per process."""
    if "nc" in _PROGRAM_CACHE:
        return _PROGRAM_CACHE["nc"]
    import concourse.bacc as bacc
    import concourse.tile as tile
    import concourse.mybir as mybir

    bf16 = mybir.dt.bfloat16
    f32 = mybir.dt.float32
    nc = bacc.Bacc("TRN2", target_bir_lowering=False, debug=False)
    aps = {
        "qT": nc.dram_tensor("qT", [D, S], bf16, kind="ExternalInput").ap(),
        "kT": nc.dram_tensor("kT", [D, S], bf16, kind="ExternalInput").ap(),
        "vT": nc.dram_tensor("vT", [D, S], bf16, kind="ExternalInput").ap(),
        "wq": nc.dram_tensor("wq", [D, COLS], bf16, kind="ExternalInput").ap(),
        "wo": nc.dram_tensor("wo", [COLS, D], bf16, kind="ExternalInput").ap(),
        "out": nc.dram_tensor("out", [S, D], f32, kind="ExternalOutput").ap(),
    }
    with tile.TileContext(nc) as tc:
        _emit_kernel(tc, aps)
    nc.compile()
    _PROGRAM_CACHE["nc"] = nc
    return nc


def make_in_maps(q, k, v, Wq, Wo):
    """Host-side sharding: core c -> batch c//2, head-half c%2."""
    q = np.asarray(q, dtype=np.float32)
    k = np.asarray(k, dtype=np.float32)
    v = np.asarray(v, dtype=np.float32)
    Wq = np.asarray(Wq, dtype=np.float32)
    Wo = np.asarray(Wo, dtype=np.float32)
    WqT = np.ascontiguousarray(Wq.T)   # (in D, out D)
    WoT = np.ascontiguousarray(Wo.T)   # (in D, out D)
    in_maps = []
    for c in range(N_CORES):
        b, half = divmod(c, 2)
        cols = slice(half * COLS, (half + 1) * COLS)
        in_maps.append({
            "qT": np.ascontiguousarray(q[b].T).astype(BF16),
            "kT": np.ascontiguousarray(k[b].T).astype(BF16),
            "vT": np.ascontiguousarray(v[b].T).astype(BF16),
            "wq": np.ascontiguousarray(WqT[:, cols]).astype(BF16),
            "wo": np.ascontiguousarray(WoT[cols, :]).astype(BF16),
        })
    return in_maps


def run_cores(in_maps, trace=False, trace_cores=None):
    from concourse.bass_utils import run_bass_kernel_spmd
    nc = build_program()
    return run_bass_kernel_spmd(
        nc, in_maps, core_ids=list(range(N_CORES)),
        trace=trace, trace_cores=trace_cores)


def kernel(q, k, v, Wq, Wo):
    in_maps = make_in_maps(q, k, v, Wq, Wo)
    res = run_cores(in_maps)
    B = 4
    out = np.zeros((B, S, D), dtype=np.float32)
    for c in range(N_CORES):
        out[c // 2] += res.results[c]["out"]
    return out


# revision 27
# speedup vs baseline: 1.5275x; 1.0128x over previous
"""Multi-head attention (shared Wq for Q/K/V projections, Wo output proj)
as a Bass/Tile kernel for 8 Trainium2 NeuronCores.

Problem: B=4, S=2048, D=1024, H=16 heads (dk=64).
  Q = q @ Wq.T ; K = k @ Wq.T ; V = v @ Wq.T   (faithful: Wq for all three)
  out = softmax(Q K^T / 8) V  -> merge heads -> @ Wo.T

Sharding: core c handles batch b=c//2 and head-half half=c%2 (8 heads = 512
projection columns). Each core computes a partial (S, D) output
(head_out_slice @ Wo.T rows) in fp32; host sums the two halves per batch.

Per-core device pipeline (all matmuls bf16, fp32 PSUM accumulate):
  P1 projections:  QT/KT (dims x seq, per head-pair tiles) and V (seq x dims,
                   with a ones column appended per head for softmax denoms).
  P2 attention per (query-block of 512, head-pair):
       MM1: ST units (128 keys, 512 q) = K^T-chunk.T @ Q^T, two heads
            row-packed in the 128x128 PE array (contraction dk=64 each).
       ACT: PT = exp(ST * 1/8) PSUM->SBUF bf16 in N=1536/1024 groups.
       MM2: accum (65, 512) += V_aug[kc].T-style lhsT (128 keys, 64+1) @ PT;
            row 64 (ones column) accumulates the softmax denominator.
       tail: reciprocal of denom row, DMA partition-broadcast (via DRAM
             bounce), normalize rows 0..63 -> head_outT bf16.
  P3 output projection per query-block: accumulate over 4 head-pairs,
     evict fp32, DMA to DRAM.
"""

import numpy as np
import ml_dtypes

BF16 = ml_dtypes.bfloat16

S = 2048          # sequence length
D = 1024          # model dim
COLS = 512        # projection columns per core (8 heads * 64)
P = 128           # SBUF partitions
DK = 64           # head dim
PAIRS = 4         # head pairs per core
KC = S // P       # 16 key chunks
RC = D // P       # 8 contraction chunks for projections
QB = 512          # query block size
NQB = S // QB     # 4 query blocks
N_CORES = 8

_PROGRAM_CACHE = {}


def _emit_kernel(tc, aps):
    import concourse.mybir as mybir

    nc = tc.nc
    f32 = mybir.dt.float32
    bf16 = mybir.dt.bfloat16
    Exp = mybir.ActivationFunctionType.Exp
    mult = mybir.AluOpType.mult

    qT, kT, vT, wq, wo, out = (
        aps["qT"], aps["kT"], aps["vT"], aps["wq"], aps["wo"], aps["out"])

    # head_outT per (pair, qcb): (128 pair-dims, 512 q)
    HOUT = [[None] * NQB for _ in range(PAIRS)]

    with (
        tc.tile_pool(name="persist", bufs=1) as persist,
        tc.tile_pool(name="stage", bufs=4) as stage,        # 4 x 16KB/part
        tc.tile_pool(name="ptp", bufs=8) as ptp,            # exp outputs
        tc.tile_pool(name="hop", bufs=12) as hop,  # head_outT tiles
        tc.tile_pool(name="smalls", bufs=2) as smalls,
        tc.tile_pool(name="osbp", bufs=2) as osbp,
        tc.tile_pool(name="dramp", bufs=4, space="DRAM") as dramp,
        tc.tile_pool(name="stps", bufs=1, space="PSUM") as stps,
        tc.tile_pool(name="pbp", bufs=3, space="PSUM") as pbp,
    ):
        # ---------------- persistent SBUF tiles ----------------
        def ptile(shape, name):
            return persist.tile(shape, bf16, tag=name, name=name)

        wq_sb = ptile([P, RC, COLS], "wq_sb")                   # 8 KB/part
        wo_sb = ptile([P, PAIRS, D], "wo_sb")                   # 8 KB/part
        QT = [ptile([P, S], f"QT{p}") for p in range(PAIRS)]
        KT = [ptile([P, S], f"KT{p}") for p in range(PAIRS)]
        # V with ones column per head: (seq part, 8 heads, 64+1)
        V = [ptile([P, 8, DK + 1], f"V{kc}") for kc in range(KC)]

        nc.sync.dma_start(wq_sb[:], wq.rearrange("(r p) n -> p r n", p=P))
        nc.sync.dma_start(wo_sb[:], wo.rearrange("(c p) n -> p c n", p=P))

        # ---------------- P1: projections (chunked staging) ----------------
        # Stage (128, 8, 512) column-chunks of the transposed inputs on
        # demand. Only V, KT[0] and QT[0][qc0] are projected up front; the
        # remaining 27 projection tiles are deadline-ordered background work
        # pumped into the attention stream (PE has slack under the ACT-bound
        # exp pipeline).
        kch = [None] * NQB
        qch = [None] * NQB

        def chunk(cache, src, qc, tagn, bufs=4):
            if cache[qc] is None:
                t = stage.tile([P, RC, QB], bf16, tag=tagn, bufs=bufs,
                               name=f"{tagn}{qc}")
                nc.sync.dma_start(
                    t[:],
                    src.rearrange("(r p) n -> p r n", p=P)[:, :, qc * QB:(qc + 1) * QB])
                cache[qc] = t
            return cache[qc]

        def proj_psum(lhsT_of_rc, rhs_of_rc, n_free):
            ps = pbp.tile([P, QB], f32, tag="pb", name="projps")
            for rc in range(RC):
                nc.tensor.matmul(
                    ps[:, :n_free], lhsT_of_rc(rc), rhs_of_rc(rc),
                    start=(rc == 0), stop=(rc == RC - 1))
            return ps

        vch = [None] * NQB

        def proj_v_tile(kc):
            qc, k4 = divmod(kc, 4)
            vc = chunk(vch, vT, qc, "vc", bufs=2)
            ps = proj_psum(
                lambda rc, k4=k4: vc[:, rc, k4 * P:(k4 + 1) * P],
                lambda rc: wq_sb[:, rc, :],
                COLS)
            nc.vector.tensor_copy(
                out=V[kc][:, :, 0:DK],
                in_=ps.rearrange("p (h d) -> p h d", d=DK))
            nc.vector.memset(V[kc][:, :, DK:DK + 1], 1.0)

        def proj_k(pair, qc):
            t = chunk(kch, kT, qc, "kc")
            ps = proj_psum(
                lambda rc: wq_sb[:, rc, pair * P:(pair + 1) * P],
                lambda rc: t[:, rc, :],
                QB)
            nc.vector.tensor_copy(
                out=KT[pair][:, qc * QB:(qc + 1) * QB], in_=ps[:])

        def proj_q(pair, qc):
            t = chunk(qch, qT, qc, "qc")
            ps = proj_psum(
                lambda rc: wq_sb[:, rc, pair * P:(pair + 1) * P],
                lambda rc: t[:, rc, :],
                QB)
            nc.vector.tensor_copy(
                out=QT[pair][:, qc * QB:(qc + 1) * QB], in_=ps[:])

        # head: V, KT[0], QT[0][qc0]
        for kc in range(KC):
            proj_v_tile(kc)
        for qc in range(NQB):
            proj_k(0, qc)
        proj_q(0, 0)

        # background projection items, ordered by first-use deadline.
        # NOTE: V must NOT go here — iteration i's own MM2 consumes V, so a
        # V-proj PSUM allocation that waits on iteration i's accumulator slot
        # release forms a dependency cycle (observed as first-exec NaN).
        bg = []
        for it in ["K1q0", "Q1q0", "K1q1", "K2q0", "K1q2", "Q2q0", "K1q3",
                   "K2q1", "K3q0", "K2q2", "Q3q0", "K2q3", "K3q1", "K3q2",
                   "K3q3"][:]:
            fn = proj_k if it[0] == "K" else proj_q
            bg.append(lambda fn=fn, p=int(it[1]), q=int(it[3]): fn(p, q))
        for qc in range(1, NQB):
            for pair in range(PAIRS):
                bg.append(lambda p=pair, q=qc: proj_q(p, q))

        def pump_bg():
            if bg:
                bg.pop(0)()

        # ---------------- P2+P3: attention + output projection ----------------
        # Software pipeline: MM2 consumption trails MM1/exp production by TD
        # groups (globally, across iteration boundaries), so the in-order PE
        # stream never blocks on a just-issued exp. Output-projection work is
        # emitted in small chunks between groups to avoid starving ACT.
        TD = 4

        def emit_tail(pair, qcb, accum):
            # Evict accumulators to SBUF immediately: frees the PSUM slots.
            raw = [smalls.tile([DK + 1, QB], f32, tag="raw", bufs=4,
                               name=f"raw{j}") for j in (0, 1)]
            for j in (0, 1):
                nc.vector.tensor_copy(out=raw[j][:], in_=accum[j][:])
            # Normalize by softmax denominator (raw row 64), off critical
            # path. DVE reciprocal on a (1, 512) AP would use one lane
            # (~3.3us); bounce through DRAM and reload as (128, 2, 4) so all
            # 128 lanes divide in parallel.
            rdram = dramp.tile([2, QB], f32, name="rdram")
            for j in (0, 1):
                nc.sync.dma_start(rdram[j:j + 1, :], raw[j][DK:DK + 1, :])
            rs = smalls.tile([P, 2, QB // P], f32, tag="rs", name="rs")
            nc.sync.dma_start(rs[:], rdram.rearrange("j (p f) -> p j f", p=P))
            rr = smalls.tile([P, 2, QB // P], f32, tag="rr", name="rr")
            nc.vector.reciprocal(rr[:], rs[:])
            rdram2 = dramp.tile([2, QB], f32, name="rdram2")
            nc.sync.dma_start(rdram2.rearrange("j (p f) -> p j f", p=P), rr[:])
            bcast = [smalls.tile([DK, QB], f32, tag="bcast", bufs=4,
                                 name=f"bcast{j}") for j in (0, 1)]
            for j in (0, 1):
                nc.sync.dma_start(
                    bcast[j][:], rdram2[j:j + 1, :].to_broadcast((DK, QB)))
            ht = hop.tile([P, QB], bf16, tag="hout", name=f"ht{pair}_{qcb}")
            for j in (0, 1):
                nc.vector.tensor_tensor(
                    ht[j * DK:(j + 1) * DK, :],
                    raw[j][0:DK, :],
                    bcast[j][:],
                    mult)
            HOUT[pair][qcb] = ht

        def outproj_qk(qcb, qk):
            q0 = qcb * QB
            osb = osbp.tile([P, D], f32, tag="osb", name="osb")
            for nk in range(2):
                ps = pbp.tile([P, QB], f32, tag="pb", name="ops")
                for pair in range(PAIRS):
                    nc.tensor.matmul(
                        ps[:],
                        HOUT[pair][qcb][:, qk * P:(qk + 1) * P],
                        wo_sb[:, pair, nk * QB:(nk + 1) * QB],
                        start=(pair == 0), stop=(pair == PAIRS - 1))
                nc.vector.tensor_copy(out=osb[:, nk * QB:(nk + 1) * QB], in_=ps[:])
            nc.sync.dma_start(
                out[q0 + qk * P: q0 + (qk + 1) * P, :], osb[:])

        pending = []       # deferred MM2 units: (pair, qcb, pt, u, j, kc, last)
        iter_accum = {}    # (pair, qcb) -> [accum0, accum1]
        oproj_items = []   # deferred outproj chunks: (ready_at_gc, qcb, qk)
        gc_box = [0]       # global group counter

        def flush_mm2_unit():
            pair, qcb, pt, u, j, kc, last = pending.pop(0)
            key = (pair, qcb)
            if key not in iter_accum:
                iter_accum[key] = [
                    pbp.tile([DK + 1, QB], f32, tag="pb", name=f"acc{jj}")
                    for jj in (0, 1)]
            accum = iter_accum[key]
            nc.tensor.matmul(
                accum[j][:],
                V[kc][:, pair * 2 + j, :],
                pt[:, u * QB:(u + 1) * QB],
                start=(kc == 0), stop=(kc == KC - 1))
            if last:
                emit_tail(pair, qcb, accum)
                del iter_accum[key]
                if pair == PAIRS - 1:
                    # The normalize chain (reciprocal via DRAM bounce +
                    # broadcast) takes ~8us; don't let the in-order PE stream
                    # hit outproj matmuls before head_outT can possibly be
                    # ready, or the whole pipeline stalls head-of-line.
                    for qk in range(QB // P):
                        oproj_items.append((gc_box[0] + 10 + qk, qcb, qk))

        TDU = 10  # MM2 trails MM1/exp by ~4 groups of units

        for qcb in range(NQB):
            q0 = qcb * QB
            for pair in range(PAIRS):
                units = [(j, kc) for kc in range(KC) for j in (0, 1)]
                gi = 0
                ui = 0
                while ui < len(units):
                    cap = 3 if gi % 2 == 0 else 2
                    group = units[ui:ui + cap]
                    n = len(group)
                    tag = "stA" if gi % 2 == 0 else "stB"
                    width = 1536 if gi % 2 == 0 else 1024
                    st = stps.tile([P, width], f32, tag=tag, name="st")
                    # MM1 (row-packed head pair: j=0 rows 0-63, j=1 rows
                    # 64-127) interleaved with deferred MM2 units so MM2's
                    # long rhs streams hide MM1's exposed LDWEIGHTS.
                    for u, (j, kc) in enumerate(group):
                        nc.tensor.matmul(
                            st[:, u * QB:(u + 1) * QB],
                            KT[pair][j * DK:(j + 1) * DK, kc * P:(kc + 1) * P],
                            QT[pair][j * DK:(j + 1) * DK, q0:q0 + QB],
                            start=True, stop=True)
                    while len(pending) > TDU:
                        flush_mm2_unit()
                    pt = ptp.tile([P, 1536], bf16, tag="pt", name="pt")
                    nc.scalar.activation(
                        pt[:, :n * QB], st[:, :n * QB], Exp, scale=0.125)
                    for u, (j, kc) in enumerate(group):
                        pending.append(
                            (pair, qcb, pt, u, j, kc,
                             ui + u + 1 == len(units)))
                    gc_box[0] += 1
                    if oproj_items and oproj_items[0][0] <= gc_box[0]:
                        _, oq, ok = oproj_items.pop(0)
                        outproj_qk(oq, ok)
                    elif gi % 2 == 1:
                        pump_bg()
                    ui += n
                    gi += 1
        while pending:
            flush_mm2_unit()
        while oproj_items:
            _, oq, ok = oproj_items.pop(0)
            outproj_qk(oq, ok)


def build_program():
    """Build + compile the single-core SPMD Bass program. Cached per process."""
    if "nc" in _PROGRAM_CACHE:
        return _PROGRAM_CACHE["nc"]
    import concourse.bacc as bacc
    import concourse.tile as tile
    import concourse.mybir as mybir

    bf16 = mybir.dt.bfloat16
    f32 = mybir.dt.float32
    nc = bacc.Bacc("TRN2", target_bir_lowering=False, debug=False)
    aps = {
        "qT": nc.dram_tensor("qT", [D, S], bf16, kind="ExternalInput").ap(),
        "kT": nc.dram_tensor("kT", [D, S], bf16, kind="ExternalInput").ap(),
        "vT": nc.dram_tensor("vT", [D, S], bf16, kind="ExternalInput").ap(),
        "wq": nc.dram_tensor("wq", [D, COLS], bf16, kind="ExternalInput").ap(),
        "wo": nc.dram_tensor("wo", [COLS, D], bf16, kind="ExternalInput").ap(),
        "out": nc.dram_tensor("out", [S, D], f32, kind="ExternalOutput").ap(),
    }
    with tile.TileContext(nc) as tc:
        _emit_kernel(tc, aps)
    nc.compile()
    _PROGRAM_CACHE["nc"] = nc
    return nc


def make_in_maps(q, k, v, Wq, Wo):
    """Host-side sharding: core c -> batch c//2, head-half c%2."""
    q = np.asarray(q, dtype=np.float32)
    k = np.asarray(k, dtype=np.float32)
    v = np.asarray(v, dtype=np.float32)
    Wq = np.asarray(Wq, dtype=np.float32)
    Wo = np.asarray(Wo, dtype=np.float32)
    WqT = np.ascontiguousarray(Wq.T)   # (in D, out D)
    WoT = np.ascontiguousarray(Wo.T)   # (in D, out D)
    in_maps = []
    for c in range(N_CORES):
        b, half = divmod(c, 2)
        cols = slice(half * COLS, (half + 1) * COLS)
        in_maps.append({
            "qT": np.ascontiguousarray(q[b].T).astype(BF16),
            "kT": np.ascontiguousarray(k[b].T).astype(BF16),
            "vT": np.ascontiguousarray(v[b].T).astype(BF16),
            "wq": np.ascontiguousarray(WqT[:, cols]).astype(BF16),
            "wo": np.ascontiguousarray(WoT[cols, :]).astype(BF16),
        })
    return in_maps


def run_cores(in_maps, trace=False, trace_cores=None):
    from concourse.bass_utils import run_bass_kernel_spmd
    nc = build_program()
    return run_bass_kernel_spmd(
        nc, in_maps, core_ids=list(range(N_CORES)),
        trace=trace, trace_cores=trace_cores)


def kernel(q, k, v, Wq, Wo):
    in_maps = make_in_maps(q, k, v, Wq, Wo)
    res = run_cores(in_maps)
    B = 4
    out = np.zeros((B, S, D), dtype=np.float32)
    for c in range(N_CORES):
        out[c // 2] += res.results[c]["out"]
    return out
